# revision 32
# baseline (speedup 1.0000x reference)
"""DOFENTransformer Trainium2 kernel, v3.

Data-parallel: 16 batches / 8 cores = one batch-PAIR per core. The
per-token attention-output rows (x_in = attn_out + residual) are built on
the TensorEngine:

  x_inT[h, q] = T1^T @ SS1 + T2^T @ rhs2 - ones^T @ mu_row
    SS1[c | c+64, q] = one-hot scatter of softmax coefficients (m, a),
    built with 7 wide tensor_tensor ops per branch from host one-hot masks
    and DMA-broadcast coefficient rows; rhs2 carries the x0/residual
    terms. mu (row means) comes from host-gathered table means, so x_in
    arrives mean-centered and LayerNorm needs only a sum of squares.

v3 changes vs v2: DMA plumbing batched (one rowt DRAM tensor, 4+2
coefficient-row exports, one import per (br,t) covering both halves,
host tables packed into 5 wide params) so queue-issue time and
first-coefficient latency shrink; m01er built with one strided
tensor_tensor per batch; wrhs2 on DVE.

Column q of x_inT holds rODT seq q. Coefficient-side tiles use slot
(p, k) = (q // 13, q % 13); x_inT is DMA-XBAR-transposed to seq-slot
layout (p', kk) = (q % 128, q // 128) for the forest contraction (PE) and
stats (strided reduces). Host tables are gathered per-slot so the two
layouts never mix on device. bf16 for all wide ops; fp32 stats + tail.
"""
import sys

for p in ('/opt/trn_rl_repo', '/root/.axon_site/_ro/trn_rl_repo'):
    if p not in sys.path:
        sys.path.insert(0, p)

import numpy as np
import ml_dtypes
import concourse.bass as bass
import concourse.bacc as bacc_mod
from concourse import mybir
from concourse.tile import TileContext
from concourse.bass_utils import run_bass_kernel_spmd

B, N_COL, N_COND, D, H = 16, 100, 64, 4, 128
N_FOREST, N_CLASS = 100, 10
NSEQ, NBLK, PAD = 1600, 13, 1664
W2 = 2 * PAD  # batch-pair width 3328
EPS = 1e-5
S128 = float(np.sqrt(128.0))
F32 = mybir.dt.float32
BF16 = mybir.dt.bfloat16
AF = mybir.ActivationFunctionType
OP = mybir.AluOpType
NCORES = 8
BF = ml_dtypes.bfloat16

CHUNK = 512
CHUNKS = [(c, min(c + CHUNK, W2)) for c in range(0, W2, CHUNK)]

# rowt rows: M(br,t)=br*8+t, A(br,t)=br*8+4+t, x0=16, mu0=17, mu1=18
ROW_M = lambda br, t: br * 8 + t
ROW_A = lambda br, t: br * 8 + 4 + t
ROW_X0 = 16
ROW_MU = lambda br: 17 + br
N_ROWS = 19

# fp32 pack layouts
P1T_GVH, P1T_XD, P1T_W = 0, 416, 546
C2_GMAW, C2_GMCW, C2_GMAE, C2_GMCE = 0, 104, 208, 312
C2_GM0, C2_GMB, C2_TAIL, C2_W = 416, 442, 468, 468 + 676
# bf16 pack layout (cbf)
CB_T1W, CB_T1E, CB_T2, CB_M01, CB_WOW = 0, 128, 256, 384, 1684
CB_NEG = 1684 + NBLK * 128
CB_W = CB_NEG + 128


def _host_precompute(inp):
    sl = lambda i: slice(i * H, (i + 1) * H)
    Wn = inp['W_num'].reshape(N_COND, H).astype(np.float32)
    Bn = inp['b_num'].reshape(N_COND, H).astype(np.float32)
    Wqkv, bqkv = inp['Wqkv'].astype(np.float32), inp['bqkv'].astype(np.float32)
    perm = inp['perm'].astype(np.int64)
    A = Wn @ Wqkv
    C = Bn @ Wqkv + bqkv

    seq = np.arange(NSEQ)
    g = seq // 64
    j = seq % 64
    p_t = np.zeros((PAD, D), np.int64)
    for t in range(D):
        p_t[:NSEQ, t] = perm[4 * g + t, j]
    valid = np.arange(PAD) < NSEQ
    # coefficient-side slots: q = p*13 + k
    q_of = np.arange(PAD)
    cs_p, cs_k = q_of // NBLK, q_of % NBLK
    out = {}

    # logits Gram tables: gvh[p, br*208 + kind*52 + k*4 + t] for q=p*13+k
    gv = np.zeros((128, 416), np.float32)
    for br in range(2):
        Aq, Ak = A[:, sl(3 * br)], A[:, sl(3 * br + 1)]
        Cq, Ck = C[:, sl(3 * br)], C[:, sl(3 * br + 1)]
        Gt = (Aq @ Ak.T, Aq @ Ck.T, Cq @ Ak.T, Cq @ Ck.T)
        for kind in range(4):
            for t in range(D):
                v = np.zeros(PAD, np.float32)
                v[valid] = Gt[kind][p_t[valid, 0], p_t[valid, t]]
                gv[cs_p, br * 208 + kind * 52 + cs_k * 4 + t] = v
    out['gvh'] = gv

    # one-hot scatter-transpose masks [128, 3328], col = b*1664 + q
    oht = {}
    for t in range(D):
        oh = np.zeros((N_COND, PAD), np.float32)
        for s in range(NSEQ):
            oh[p_t[s, t], s] += 1.0
        ohd = np.concatenate([oh, oh], 0)
        oht[t] = np.tile(ohd, (1, 2)).astype(BF)
    for t in range(D):
        out[f'oht{t}'] = oht[t]
    oh0 = np.zeros((N_COND, PAD), np.float32)
    for s in range(NSEQ):
        oh0[p_t[s, 0], s] += 1.0
    out['oht0t'] = np.tile(oh0, (1, 2)).astype(BF)  # [64, 3328]

    Av, Cv = {}, {}
    for br in range(2):
        WV, bV = Wqkv[:, sl(3 * br + 2)], bqkv[sl(3 * br + 2)]
        Av[br] = Wn @ WV
        Cv[br] = Bn @ WV + bV
    t1w = np.concatenate([Av[0], Cv[0]], 0).astype(BF)
    t1e = np.concatenate([Av[1], Cv[1]], 0).astype(BF)
    t2 = np.concatenate([Wn, Bn], 0).astype(BF)

    # gathered row-mean tables, coefficient slots [p, b*52 + k*4 + t]
    gm = {}
    for br in range(2):
        ta = np.zeros((128, 104), np.float32)
        tcn = np.zeros((128, 104), np.float32)
        rmA, rmC = Av[br].mean(1), Cv[br].mean(1)
        for t in range(D):
            va = np.zeros(PAD, np.float32)
            vc = np.zeros(PAD, np.float32)
            va[valid] = rmA[p_t[valid, t]]
            vc[valid] = rmC[p_t[valid, t]]
            for b in range(2):
                ta[cs_p, b * 52 + cs_k * 4 + t] = va
                tcn[cs_p, b * 52 + cs_k * 4 + t] = vc
        gm[(br, 'a')] = ta
        gm[(br, 'c')] = tcn
    g0 = np.zeros((128, 26), np.float32)
    gB = np.zeros((128, 26), np.float32)
    rmW, rmB = Wn.mean(1), Bn.mean(1)
    v0 = np.zeros(PAD, np.float32)
    vB = np.zeros(PAD, np.float32)
    v0[valid] = rmW[p_t[valid, 0]]
    vB[valid] = rmB[p_t[valid, 0]]
    for b in range(2):
        g0[cs_p, b * 13 + cs_k] = v0
        gB[cs_p, b * 13 + cs_k] = vB

    Wowg = inp['gamma_w'].astype(np.float32) * inp['Wow'][:, 0].astype(np.float32)
    wowgr = np.tile(Wowg[None, :], (128, NBLK)).astype(BF)

    # forest mask in transposed slots: m01[p', kk*100+f] = M01[kk*128+p', f]
    swr = inp['swr'].astype(np.int64)
    M01 = np.zeros((PAD, N_FOREST), np.float32)
    for f in range(N_FOREST):
        r = swr[f]
        s = (r % 25) * 64 + (r // 25)
        M01[s, f] = 1.0
    m01 = M01.reshape(NBLK, 128, N_FOREST).transpose(1, 0, 2).reshape(128, NBLK * N_FOREST).astype(BF)

    tailc = np.zeros((128, 676), np.float32)
    tailc[0:1, 420:548] = 1.0
    tailc[0:1, 548:676] = -1.0
    tailc[:, 0:128] = inp['gamma_E'].astype(np.float32)[:, None] * inp['WoE'].astype(np.float32)
    tailc[:, 128:256] = inp['g1'].astype(np.float32)[:, None] * inp['W1'].astype(np.float32)
    W2p = inp['g2'].astype(np.float32)[:, None] * inp['W2'].astype(np.float32)
    tailc[:, 256:272] = np.concatenate([W2p, np.zeros((H, 6), np.float32)], 1)
    tailc[:, 272:273] = (inp['be1'] @ inp['W1'] + inp['b1'])[:, None].astype(np.float32)
    tailc[:, 273:274] = 1.0
    tailc[:, 274:275] = EPS
    tailc[:, 275:276] = float(inp['beta_w'] @ inp['Wow'][:, 0] + inp['bow'][0])
    b2p = (inp['be2'] @ inp['W2'] + inp['b2']).astype(np.float32)
    tailc[0:16, 276:277] = np.concatenate([b2p, np.zeros(6, np.float32)])[:, None]
    tailc[0:1, 277:405] = (inp['beta_E'] @ inp['WoE'] + inp['boE'])[None, :].astype(np.float32)

    # fp32 pack 2
    cst2 = np.zeros((128, C2_W), np.float32)
    cst2[:, C2_GMAW:C2_GMAW + 104] = gm[(0, 'a')]
    cst2[:, C2_GMCW:C2_GMCW + 104] = gm[(0, 'c')]
    cst2[:, C2_GMAE:C2_GMAE + 104] = gm[(1, 'a')]
    cst2[:, C2_GMCE:C2_GMCE + 104] = gm[(1, 'c')]
    cst2[:, C2_GM0:C2_GM0 + 26] = g0
    cst2[:, C2_GMB:C2_GMB + 26] = gB
    cst2[:, C2_TAIL:C2_TAIL + 676] = tailc
    out['cst2'] = cst2

    # bf16 pack
    cbf = np.zeros((128, CB_W), BF)
    cbf[:, CB_T1W:CB_T1W + 128] = t1w
    cbf[:, CB_T1E:CB_T1E + 128] = t1e
    cbf[:, CB_T2:CB_T2 + 128] = t2
    cbf[:, CB_M01:CB_M01 + NBLK * 100] = m01
    cbf[:, CB_WOW:CB_WOW + NBLK * 128] = wowgr
    cbf[0:1, CB_NEG:CB_NEG + 128] = -1.0
    cbf[32:33, CB_NEG:CB_NEG + 128] = -1.0
    out['cbf'] = cbf
    return out


def _host_x(inp, bs, gvh):
    x = inp['x'].astype(np.float32)
    q_of = np.arange(NSEQ)
    cs_p, cs_k = q_of // NBLK, q_of % NBLK
    g = q_of // 64
    p1t = np.zeros((128, P1T_W), np.float32)
    p1t[:, P1T_GVH:P1T_GVH + 416] = gvh
    for bi, b in enumerate(bs):
        for t in range(D):
            p1t[cs_p, P1T_XD + bi * 52 + cs_k * 4 + t] = x[b, 4 * g + t]
        p1t[cs_p, P1T_XD + 104 + bi * 13 + cs_k] = x[b, 4 * g]
    return {'p1t': p1t}


_H_SHAPES = {
    'p1t': ((128, P1T_W), F32),
    'cst2': ((128, C2_W), F32),
    'oht0': ((128, W2), BF16), 'oht1': ((128, W2), BF16),
    'oht2': ((128, W2), BF16), 'oht3': ((128, W2), BF16),
    'oht0t': ((64, W2), BF16),
    'cbf': ((128, CB_W), BF16),
}


def _vw(ap, off, dims):
    return bass.AP(tensor=ap.tensor, offset=ap.offset + off,
                   ap=[list(ap.ap[0])] + [[s, c] for (s, c) in dims])


def _vw1(ap):
    return bass.AP(tensor=ap.tensor, offset=ap.offset,
                   ap=[[ap.ap[0][0], 1]] + [list(d) for d in ap.ap[1:]])


def _dram_ap(handle, off, dims):
    return bass.AP(tensor=handle.tensor, offset=handle.offset + off,
                   ap=[[s, c] for (s, c) in dims])


DEBUG = False


def _build_nc():
    nc = bacc_mod.Bacc()
    dram = {k: nc.declare_dram_parameter(k, list(sh), dt, isOutput=False)
            for k, (sh, dt) in _H_SHAPES.items()}
    out_d = nc.declare_dram_parameter('out', [2, 16], F32, isOutput=True)
    if DEBUG:
        dbg = {
            'd_mbf': nc.declare_dram_parameter('d_mbf', [128, 208], BF16, isOutput=True),
            'd_ct00': nc.declare_dram_parameter('d_ct00', [128, W2], BF16, isOutput=True),
            'd_ct11': nc.declare_dram_parameter('d_ct11', [128, W2], BF16, isOutput=True),
            'd_ss0': nc.declare_dram_parameter('d_ss0', [128, W2], BF16, isOutput=True),
            'd_xs0': nc.declare_dram_parameter('d_xs0', [128, W2], BF16, isOutput=True),
            'd_mur': nc.declare_dram_parameter('d_mur', [2, W2], BF16, isOutput=True),
            'd_wr2': nc.declare_dram_parameter('d_wr2', [128, W2], BF16, isOutput=True),
            'd_muf': nc.declare_dram_parameter('d_muf', [128, 52], F32, isOutput=True),
        }
    rowt = nc.dram_tensor('rowt', [N_ROWS, W2], BF16)
    rbase = rowt[0, 0:1]

    with TileContext(nc) as tc:
        with (
            tc.tile_pool(name='const', bufs=1) as cp,
            tc.tile_pool(name='work', bufs=1) as wk,
            tc.tile_pool(name='cpT', bufs=5) as cpT,
            tc.tile_pool(name='scr2', bufs=3) as scr2,
            tc.tile_pool(name='px', bufs=2, space='PSUM') as px,
            tc.tile_pool(name='pt', bufs=2, space='PSUM') as pt_pool,
            tc.tile_pool(name='pz', bufs=2, space='PSUM') as pz,
        ):
            tl = {}
            # critical P1/P2 tables first on sync with the full DMA bus; the
            # big mask loads are gated behind p1t arrival via a tiny WAW
            # write into each destination tile so they can't contend
            for k, q in (('p1t', nc.sync), ('cst2', nc.sync)):
                sh, dt = _H_SHAPES[k]
                t = cp.tile(list(sh), dt, name=f'c_{k}')
                q.dma_start(out=t[...], in_=dram[k][...])
                tl[k] = t
            touch = cp.tile([128, 1], F32, tag='touch')

            def _touch(src, eng=None):
                (eng or nc.vector).tensor_copy(touch[0:src.ap[0][1], 0:1], src)

            wrhs2 = wk.tile([128, W2], BF16, tag='wrhs2')
            for k, q in (('oht0', nc.scalar), ('oht2', nc.gpsimd),
                         ('oht1', nc.scalar), ('oht3', nc.gpsimd),
                         ('oht0t', nc.scalar), ('cbf', nc.gpsimd)):
                sh, dt = _H_SHAPES[k]
                t = cp.tile(list(sh), dt, name=f'c_{k}')
                nc.gpsimd.tensor_copy(t[0:1, 0:1], tl['p1t'][0:1, 0:1])
                q.dma_start(out=t[...], in_=dram[k][...])
                tl[k] = t
            nc.gpsimd.tensor_copy(wrhs2[64:65, 0:1], tl['p1t'][0:1, 0:1])
            nc.scalar.dma_start(out=wrhs2[64:128, :], in_=dram['oht0t'][...])

            cst2 = tl['cst2']
            tailc = cst2[:, C2_TAIL:C2_TAIL + 676]
            WoEg = tailc[:, 0:128]
            W1p = tailc[:, 128:256]
            W2pc = tailc[:, 256:272]
            b1p = tailc[:, 272:273]
            ones = tailc[:, 273:274]
            eps_sb = tailc[:, 274:275]
            bow2 = tailc[:, 275:276]
            b2p = tailc[0:16, 276:277]
            boE2 = tailc[0:1, 277:405]
            onesrow = tailc[0:1, 420:548]
            cbf = tl['cbf']
            negrow = cbf[0:1, CB_NEG:CB_NEG + 128]
            oht = {t: tl[f'oht{t}'][:, :] for t in range(D)}

            _touch(tl['p1t'][:, 0:1])
            _touch(cst2[:, 0:1])

            # ---------------- P1: softmax per batch -> m/a ----------------
            m_all = wk.tile([128, 208], F32, tag='m_all')
            a_all = wk.tile([128, 208], F32, tag='a_all')
            # bf16, export layout col = br*104 + t*26 + b*13 + k
            m_bf = wk.tile([128, 208], BF16, tag='m_bf')
            a_bf = wk.tile([128, 208], BF16, tag='a_bf')
            x0_bf = wk.tile([128, 26], BF16, tag='x0_bf')
            xt_rows = tl['p1t'][:, P1T_XD:P1T_XD + 104]
            x0_rows = tl['p1t'][:, P1T_XD + 104:P1T_XD + 130]
            nc.scalar.copy(x0_bf[:, :], x0_rows)
            gvv = lambda kind: _vw(tl['p1t'][:, :], P1T_GVH + kind * 52,
                                   [(208, 2), (4, NBLK), (1, 4)])
            for b in range(2):
                xtv = _vw(xt_rows, b * 52, [(0, 2), (4, NBLK), (1, 4)])
                x0v = _vw(x0_rows, b * NBLK, [(0, 2), (1, NBLK), (0, 4)])
                t1 = wk.tile([128, 104], F32, name=f't1_{b}')
                t2 = wk.tile([128, 104], F32, name=f't2_{b}')
                nc.vector.tensor_mul(t1[:, :], gvv(0), xtv)
                nc.gpsimd.tensor_mul(t2[:, :], gvv(2), xtv)
                nc.vector.tensor_add(t1[:, :], t1[:, :], gvv(1))
                nc.gpsimd.tensor_add(t2[:, :], t2[:, :], gvv(3))
                nc.vector.tensor_mul(t1[:, :], t1[:, :], x0v)
                nc.vector.tensor_add(t1[:, :], t1[:, :], t2[:, :])
                e = wk.tile([128, 104], F32, name=f'e_{b}')
                nc.scalar.activation(e[:, :], t1[:, :], AF.Exp, bias=0.0, scale=S128)
                esum = wk.tile([128, 26], F32, name=f'esum_{b}')
                nc.vector.tensor_reduce(esum[:, :], _vw(e[:, :], 0, [(4, 26), (1, 4)]),
                                        mybir.AxisListType.X, OP.add)
                nc.vector.reciprocal(esum[:, :], esum[:, :])
                asl = a_all[:, b * 104:(b + 1) * 104]
                msl = m_all[:, b * 104:(b + 1) * 104]
                nc.vector.tensor_mul(asl, e[:, :], _vw(esum[:, :], 0, [(1, 26), (0, 4)]))
                nc.vector.tensor_mul(msl, asl, xtv)
                for src, dst in ((a_all, a_bf), (m_all, m_bf)):
                    nc.scalar.copy(
                        _vw(dst[:, :], b * 13, [(104, 2), (26, 4), (1, 13)]),
                        _vw(src[:, :], b * 104, [(52, 2), (1, 4), (4, 13)]))

            # ---------------- P3/P4: export rows + broadcast imports -------
            # one export DMA per (br, kind): 4 rows each; dst rows br*8+t(+4)
            # src cols t*26+b*13+k are contiguous (t,b,k) t-major, matching
            # dst dims (p, tb, k): consecutive t rows are W2 apart and b
            # halves PAD apart, so (t,b) merges to stride PAD count 8
            XSRC = [(1, 104)]
            XDST = [(NBLK, 128), (PAD, 8), (1, NBLK)]
            for br in range(2):
                for src, r0 in ((m_bf, ROW_M(br, 0)), (a_bf, ROW_A(br, 0))):
                    nc.sync.dma_start(
                        out=_dram_ap(rbase, r0 * W2, XDST),
                        in_=_vw(src[:, :], br * 104, XSRC))
            # x0 export: dst row 16, dims (p, b, k)
            nc.sync.dma_start(
                out=_dram_ap(rbase, ROW_X0 * W2, [(NBLK, 128), (PAD, 2), (1, NBLK)]),
                in_=_vw(x0_bf[:, :], 0, [(NBLK, 2), (1, NBLK)]))

            # ct imports: one DMA per (br,t), m half -> parts 0:64, a -> 64:128
            coefT = {}
            iq = {0: nc.scalar, 1: nc.gpsimd}
            for br in range(2):
                for t in range(D):
                    ct = cpT.tile([128, W2], BF16, name='ct')
                    iq[br].dma_start(
                        out=ct[0:64, :],
                        in_=_dram_ap(rbase, ROW_M(br, t) * W2, [(0, 64), (1, W2)]))
                    iq[br].dma_start(
                        out=ct[64:128, :],
                        in_=_dram_ap(rbase, ROW_A(br, t) * W2, [(0, 64), (1, W2)]))
                    coefT[(br, t)] = ct

            # ---------------- P2: mu via gathered row-means ----------------
            mu = wk.tile([128, 52], F32, tag='mu')       # col = br*26 + b*13 + k
            mu_bf = wk.tile([128, 52], BF16, tag='mu_bf')
            for br, (ga, gc) in enumerate(((C2_GMAW, C2_GMCW), (C2_GMAE, C2_GMCE))):
                pm = wk.tile([128, 104], F32, name=f'pm_{br}')
                pa = wk.tile([128, 104], F32, name=f'pa_{br}')
                mview = _vw(m_all[:, :], br * 52, [(104, 2), (1, 52)])
                aview = _vw(a_all[:, :], br * 52, [(104, 2), (1, 52)])
                nc.vector.tensor_mul(pm[:, :], mview, cst2[:, ga:ga + 104])
                nc.gpsimd.tensor_mul(pa[:, :], aview, cst2[:, gc:gc + 104])
                nc.vector.tensor_add(pm[:, :], pm[:, :], pa[:, :])
                musl = mu[:, br * 26:(br + 1) * 26]
                nc.vector.tensor_reduce(musl, _vw(pm[:, :], 0, [(52, 2), (4, 13), (1, 4)]),
                                        mybir.AxisListType.X, OP.add)
                x0g = wk.tile([128, 26], F32, name=f'x0g_{br}')
                nc.gpsimd.tensor_mul(x0g[:, :], x0_rows, cst2[:, C2_GM0:C2_GM0 + 26])
                nc.vector.tensor_add(musl, musl, x0g[:, :])
                nc.vector.tensor_add(musl, musl, cst2[:, C2_GMB:C2_GMB + 26])
                nc.scalar.copy(mu_bf[:, br * 26:(br + 1) * 26], musl)

            # mu export: both rows in one DMA; dims (p, br*b, k) — (br,b)
            # merges since mu rows are W2 apart
            nc.sync.dma_start(
                out=_dram_ap(rbase, ROW_MU(0) * W2,
                             [(NBLK, 128), (PAD, 4), (1, NBLK)]),
                in_=_vw(mu_bf[:, :], 0, [(NBLK, 4), (1, NBLK)]))

            x0bc_t = wk.tile([64, W2], BF16, tag='x0bc_t')
            x0bc = x0bc_t[:, :]
            nc.sync.dma_start(out=x0bc,
                              in_=_dram_ap(rbase, ROW_X0 * W2, [(0, 64), (1, W2)]))
            murows = wk.tile([33, W2], BF16, tag='murows')
            murow = {0: murows[0:1, :], 1: murows[32:33, :]}
            for br in range(2):
                nc.sync.dma_start(
                    out=murow[br],
                    in_=_dram_ap(rbase, ROW_MU(br) * W2, [(0, 1), (1, W2)]))

            # ---------------- P5: SS1 scatter build ------------------------
            SS1 = {}
            sstmp = {}
            for br in range(2):
                ss = wk.tile([128, W2], BF16, name=f'ss1_{br}')
                tmpa = wk.tile([128, W2], BF16, name=f'sstmpa_{br}')
                tmpb = wk.tile([128, W2], BF16, name=f'sstmpb_{br}')
                SS1[br] = ss
                sstmp[br] = (tmpa, tmpb)
            _touch(x0bc[0:1, 0:1])
            nc.gpsimd.tensor_mul(wrhs2[0:64, :], tl['oht0t'][:, :], x0bc)
            for br in range(2):
                eng = nc.vector
                ss = SS1[br]
                tmpa, tmpb = sstmp[br]
                dsts = (ss, tmpa, tmpb, tmpa)
                for t in range(D):
                    ct = coefT[(br, t)]
                    _touch(ct[:, 0:1], eng)
                    if DEBUG and (br, t) == (0, 0):
                        nc.sync.dma_start(out=dbg['d_ct00'][...], in_=ct[...])
                    if DEBUG and (br, t) == (1, 1):
                        nc.sync.dma_start(out=dbg['d_ct11'][...], in_=ct[...])
                    eng.tensor_mul(dsts[t][:, :], oht[t][:, :], ct[:, :])
                    if t == 1:
                        eng.tensor_add(ss[:, :], ss[:, :], tmpa[:, :])
                eng.tensor_add(ss[:, :], ss[:, :], tmpb[:, :])
                eng.tensor_add(ss[:, :], ss[:, :], tmpa[:, :])
            for br in range(2):
                _touch(murow[br][0:1, 0:1])

            # ---------------- P6/P7: x_inT matmuls + copy out --------------
            t1t = {0: cbf[:, CB_T1W:CB_T1W + 128], 1: cbf[:, CB_T1E:CB_T1E + 128]}
            t2c = cbf[:, CB_T2:CB_T2 + 128]
            xs = {}
            for br in range(2):
                x_s = wk.tile([128, W2], BF16, name=f'xs_{br}')
                xs[br] = x_s
                for ci, (c0, c1) in enumerate(CHUNKS):
                    pch = px.tile([128, CHUNK], F32, name='pxc')[:, 0:c1 - c0]
                    nc.tensor.matmul(pch[:, :], t1t[br][:, :], SS1[br][:, c0:c1],
                                     start=True, stop=False)
                    nc.tensor.matmul(pch[:, :], t2c, wrhs2[:, c0:c1],
                                     start=False, stop=False)
                    nc.tensor.matmul(pch[:, :],
                                     cbf[32 * br:32 * br + 1, CB_NEG:CB_NEG + 128],
                                     murow[br][0:1, c0:c1], start=False, stop=True)
                    nc.scalar.copy(x_s[:, c0:c1], pch[:, :])

            # ---------------- P8: DMA transpose to seq-slot layout ---------
            qeng = (nc.sync, nc.scalar)
            xin = {}
            for br in range(2):
                for b in range(2):
                    xt_ = wk.tile([128, NBLK, 128], BF16, name=f'xin_{br}_{b}')
                    qeng[br].dma_start(out=xt_[:, :, :],
                                       in_=xs[br][:, b * PAD:(b + 1) * PAD],
                                       transpose=True)
                    xin[(br, b)] = xt_

            # ---------------- P9: stats ------------------------------------
            veng = (nc.vector, nc.gpsimd)
            ssq = wk.tile([128, 52], F32, tag='ssq')      # col = br*26 + b*13 + kk
            wraw = wk.tile([128, 26], F32, tag='wraw')
            for br in range(2):
                for b in range(2):
                    xt_ = xin[(br, b)]
                    _touch(xt_[:, 0, 0:1], veng[br])
                    sq = scr2.tile([128, PAD], BF16, name='sqt')
                    nc.scalar.square(sq[:, :], _vw(xt_[:, :, :], 0, [(1, PAD)]))
                    nc.vector.tensor_reduce(
                        ssq[:, br * 26 + b * 13: br * 26 + (b + 1) * 13],
                        _vw(sq[:, :], 0, [(128, NBLK), (1, 128)]),
                        mybir.AxisListType.X, OP.add)
            for b in range(2):
                wx = scr2.tile([128, PAD], BF16, name='sqt')
                nc.gpsimd.tensor_mul(wx[:, :], _vw(xin[(0, b)][:, :, :], 0, [(1, PAD)]),
                                     cbf[:, CB_WOW:CB_WOW + NBLK * 128])
                nc.vector.tensor_reduce(wraw[:, b * 13:(b + 1) * 13],
                                        _vw(wx[:, :], 0, [(128, NBLK), (1, 128)]),
                                        mybir.AxisListType.X, OP.add)
            stdv = wk.tile([128, 52], F32, tag='stdv')
            nc.scalar.activation(stdv[:, :], ssq[:, :], AF.Sqrt,
                                 bias=eps_sb, scale=1.0 / H)
            rstd = wk.tile([128, 52], F32, tag='rstd')
            nc.vector.reciprocal(rstd[:, :], stdv[:, :])
            o2 = wk.tile([128, 26], F32, tag='o2')
            nc.vector.tensor_mul(o2[:, :], wraw[:, :], rstd[:, 0:26])
            expw = wk.tile([128, 26], F32, tag='expw')
            nc.scalar.activation(expw[:, :], o2[:, :], AF.Exp, bias=bow2, scale=1.0)
            er = wk.tile([128, 26], F32, tag='er')
            nc.vector.tensor_mul(er[:, :], expw[:, :], rstd[:, 26:52])
            std_bf = wk.tile([128, 26], BF16, tag='std_bf')
            nc.scalar.copy(std_bf[:, :], stdv[:, 26:52])

            # ---------------- P10: forest ----------------------------------
            # main_c = sum_seq er*E_c*M01 via 13 accumulating matmuls; the
            # forest softmax denominator reuses M01er: z = sum std_E*M01er
            m01c = cbf[:, CB_M01:CB_M01 + NBLK * 100]
            psC, psZ = {}, {}
            for b in range(2):
                m01er = wk.tile([128, NBLK * 100], BF16, name=f'm01er_{b}')
                nc.vector.tensor_mul(
                    m01er[:, :], m01c,
                    _vw(er[:, :], b * NBLK, [(1, NBLK), (0, 100)]))
                pc = pt_pool.tile([128, 200], F32, name='tailps')[:, 0:100]
                pzt = pz.tile([16, 200], F32, name='rowps')[0:1, 0:100]
                for k in range(NBLK):
                    nc.tensor.matmul(pc, xin[(1, b)][:, k, :],
                                     m01er[:, k * 100:(k + 1) * 100],
                                     start=(k == 0), stop=(k == NBLK - 1))
                    nc.tensor.matmul(pzt, std_bf[:, b * 13 + k: b * 13 + k + 1],
                                     m01er[:, k * 100:(k + 1) * 100],
                                     start=(k == 0), stop=(k == NBLK - 1),
                                     skip_group_check=True)
                psC[b], psZ[b] = pc, pzt

            # ---------------- P11: tail (batch-pair fused) -----------------
            main_s = wk.tile([128, 200], F32, tag='main_s')
            z_s = wk.tile([1, 200], F32, tag='z_s')
            for b in range(2):
                nc.scalar.copy(main_s[:, b * 100:(b + 1) * 100], psC[b])
                nc.vector.tensor_copy(z_s[0:1, b * 100:(b + 1) * 100], psZ[b])
            pt = pt_pool.tile([128, 200], F32, name='tailps')
            nc.tensor.matmul(pt[:, :], WoEg, main_s[:, :], start=True, stop=False)
            nc.tensor.matmul(pt[:, :], boE2, z_s[0:1, :], start=False, stop=True,
                             skip_group_check=True)
            rz = wk.tile([1, 200], F32, tag='rz')
            nc.vector.reciprocal(rz[:, :], z_s[:, :])
            rzb = px.tile([128, CHUNK], F32, name='pxc')[:, 0:200]
            nc.tensor.matmul(rzb, onesrow, rz[0:1, :], start=True, stop=True)
            ft_s = wk.tile([128, 200], F32, tag='ft_s')
            nc.scalar.copy(ft_s[:, :], pt[:, :])
            F_s = wk.tile([128, 200], F32, tag='F_s')
            nc.vector.tensor_mul(F_s[:, :], ft_s[:, :], rzb)

            def ln_cols(V, nm):
                cs1 = pz.tile([16, 200], F32, name='rowps')[0:1, :]
                cs2 = pz.tile([16, 200], F32, name='rowps')[0:1, :]
                sqv = wk.tile([128, 200], F32, name=f'sqv_{nm}')
                nc.gpsimd.tensor_mul(sqv[:, :], V, V)
                nc.tensor.matmul(cs1, ones, V, start=True, stop=True)
                nc.tensor.matmul(cs2, ones, sqv[:, :], start=True, stop=True)
                strow = wk.tile([1, 400], F32, name=f'st_{nm}')
                tmp = wk.tile([1, 200], F32, name=f'tmp_{nm}')
                nc.vector.tensor_scalar_mul(strow[0:1, 0:200], cs1, 1.0 / H)
                nc.vector.tensor_scalar_mul(strow[0:1, 200:400], cs2, 1.0 / H)
                nc.vector.tensor_mul(tmp[:, :], strow[0:1, 0:200], strow[0:1, 0:200])
                nc.vector.tensor_sub(strow[0:1, 200:400], strow[0:1, 200:400], tmp[:, :])
                nc.scalar.activation(strow[0:1, 200:400], strow[0:1, 200:400],
                                     AF.Sqrt, bias=_vw1(eps_sb), scale=1.0)
                nc.vector.reciprocal(strow[0:1, 200:400], strow[0:1, 200:400])
                # strow[0:200] <- mu*rstd
                nc.vector.tensor_mul(strow[0:1, 0:200], strow[0:1, 0:200],
                                     strow[0:1, 200:400])
                mb = px.tile([128, CHUNK], F32, name='pxc')[:, 0:400]
                nc.tensor.matmul(mb, onesrow, strow[0:1, :], start=True, stop=True)
                LN = wk.tile([128, 200], F32, name=f'ln_{nm}')
                nc.vector.tensor_mul(LN[:, :], V, mb[:, 200:400])
                nc.vector.tensor_sub(LN[:, :], LN[:, :], mb[:, 0:200])
                return LN

            LN1 = ln_cols(F_s[:, :], 'l1')
            pt2 = pt_pool.tile([128, 200], F32, name='tailps')
            nc.tensor.matmul(pt2[:, :], W1p, LN1[:, :], start=True, stop=True)
            h1 = wk.tile([128, 200], F32, tag='h1')
            nc.scalar.activation(h1[:, :], pt2[:, :], AF.Relu, bias=b1p, scale=1.0)
            LN2 = ln_cols(h1[:, :], 'l2')
            po = pz.tile([16, 200], F32, name='rowps')
            nc.tensor.matmul(po[0:16, :], W2pc, LN2[:, :], start=True, stop=True)
            ob = wk.tile([16, 200], F32, tag='ob')
            nc.scalar.activation(ob[:, :], po[0:16, :], AF.Identity, bias=b2p, scale=1.0)
            ored = wk.tile([16, 2], F32, tag='ored')
            nc.vector.tensor_reduce(ored[:, :], _vw(ob[:, :], 0, [(100, 2), (1, 100)]),
                                    mybir.AxisListType.X, OP.add)
            ofin = wk.tile([16, 2], F32, tag='ofin')
            nc.vector.tensor_scalar_mul(ofin[:, :], ored[:, :], 1.0 / N_FOREST)
            for b in range(2):
                nc.sync.dma_start(out=out_d[b, :], in_=ofin[:, b:b + 1])
            if DEBUG:
                nc.sync.dma_start(out=dbg['d_mbf'][...], in_=m_bf[...])
                nc.sync.dma_start(out=dbg['d_ss0'][...], in_=SS1[0][...])
                nc.sync.dma_start(out=dbg['d_muf'][...], in_=mu[...])
                nc.sync.dma_start(out=dbg['d_xs0'][...], in_=xs[0][...])
                nc.sync.dma_start(out=dbg['d_mur'][0, :], in_=murow[0])
                nc.sync.dma_start(out=dbg['d_mur'][1, :], in_=murow[1])
                nc.sync.dma_start(out=dbg['d_wr2'][...], in_=wrhs2[...])
    nc.finalize()
    return nc


_NC_CACHE = {}


def kernel(**inputs):
    inp = {k: np.asarray(v) for k, v in inputs.items()}
    H_ = _host_precompute(inp)
    if 'nc' not in _NC_CACHE:
        _NC_CACHE['nc'] = _build_nc()
    nc = _NC_CACHE['nc']
    gvh = H_.pop('gvh')
    in_maps = []
    for c in range(NCORES):
        m = {k: np.ascontiguousarray(H_[k]) for k in H_}
        m.update({k: np.ascontiguousarray(v)
                  for k, v in _host_x(inp, (2 * c, 2 * c + 1), gvh).items()})
        in_maps.append(m)
    res = run_bass_kernel_spmd(nc, in_maps, list(range(NCORES)))
    out = np.zeros((B, N_CLASS), np.float32)
    for c in range(NCORES):
        out[2 * c:2 * c + 2] = res.results[c]['out'][:, :N_CLASS]
    return out


# revision 35
# speedup vs baseline: 1.0699x; 1.0699x over previous
"""DOFENTransformer Trainium2 kernel, v3.

Data-parallel: 16 batches / 8 cores = one batch-PAIR per core. The
per-token attention-output rows (x_in = attn_out + residual) are built on
the TensorEngine:

  x_inT[h, q] = T1^T @ SS1 + T2^T @ rhs2 - ones^T @ mu_row
    SS1[c | c+64, q] = one-hot scatter of softmax coefficients (m, a),
    built with 7 wide tensor_tensor ops per branch from host one-hot masks
    and DMA-broadcast coefficient rows; rhs2 carries the x0/residual
    terms. mu (row means) comes from host-gathered table means, so x_in
    arrives mean-centered and LayerNorm needs only a sum of squares.

v3 changes vs v2: DMA plumbing batched (one rowt DRAM tensor, 4+2
coefficient-row exports, one import per (br,t) covering both halves,
host tables packed into 5 wide params) so queue-issue time and
first-coefficient latency shrink; m01er built with one strided
tensor_tensor per batch; wrhs2 on DVE.

Column q of x_inT holds rODT seq q. Coefficient-side tiles use slot
(p, k) = (q // 13, q % 13); x_inT is DMA-XBAR-transposed to seq-slot
layout (p', kk) = (q % 128, q // 128) for the forest contraction (PE) and
stats (strided reduces). Host tables are gathered per-slot so the two
layouts never mix on device. bf16 for all wide ops; fp32 stats + tail.
"""
import sys

for p in ('/opt/trn_rl_repo', '/root/.axon_site/_ro/trn_rl_repo'):
    if p not in sys.path:
        sys.path.insert(0, p)

import numpy as np
import ml_dtypes
import concourse.bass as bass
import concourse.bacc as bacc_mod
from concourse import mybir
from concourse.tile import TileContext
from concourse.bass_utils import run_bass_kernel_spmd

B, N_COL, N_COND, D, H = 16, 100, 64, 4, 128
N_FOREST, N_CLASS = 100, 10
NSEQ, NBLK, PAD = 1600, 13, 1664
W2 = 2 * PAD  # batch-pair width 3328
EPS = 1e-5
S128 = float(np.sqrt(128.0))
F32 = mybir.dt.float32
BF16 = mybir.dt.bfloat16
AF = mybir.ActivationFunctionType
OP = mybir.AluOpType
NCORES = 8
BF = ml_dtypes.bfloat16

CHUNK = 512
CHUNKS = [(c, min(c + CHUNK, W2)) for c in range(0, W2, CHUNK)]

# rowt rows: M(br,t)=br*8+t, A(br,t)=br*8+4+t, x0=16, mu0=17, mu1=18
ROW_M = lambda br, t: br * 8 + t
ROW_A = lambda br, t: br * 8 + 4 + t
ROW_X0 = 16
ROW_MU = lambda br: 17 + br
N_ROWS = 19

# fp32 pack layouts
P1T_GVH, P1T_XD, P1T_W = 0, 416, 546
C2_GMAW, C2_GMCW, C2_GMAE, C2_GMCE = 0, 104, 208, 312
C2_GM0, C2_GMB, C2_TAIL, C2_W = 416, 442, 468, 468 + 676
# bf16 pack layout (cbf)
CB_T1W, CB_T1E, CB_T2, CB_M01, CB_WOW = 0, 128, 256, 384, 1684
CB_NEG = 1684 + NBLK * 128
CB_WOE = CB_NEG + 128          # [128,128] gamma_E*WoE
CB_W1B = CB_WOE + 128          # [128,128] g1*W1
CB_W2B = CB_W1B + 128          # [128,16]  g2*W2 / n_forest
CB_ONEHC = CB_W2B + 16         # [128,1] column of 1/128
CB_ONE1 = CB_ONEHC + 1         # row(part0): ones
CB_BOE2 = CB_ONE1 + 128        # row(part0): beta_E@WoE+boE
CB_W1SN = CB_BOE2 + 128        # row(part0): -colsum(g1*W1)
CB_W2SN = CB_W1SN + 128        # row(part0): -colsum(W2B)
CB_W = CB_W2SN + 16


def _host_precompute(inp):
    sl = lambda i: slice(i * H, (i + 1) * H)
    Wn = inp['W_num'].reshape(N_COND, H).astype(np.float32)
    Bn = inp['b_num'].reshape(N_COND, H).astype(np.float32)
    Wqkv, bqkv = inp['Wqkv'].astype(np.float32), inp['bqkv'].astype(np.float32)
    perm = inp['perm'].astype(np.int64)
    A = Wn @ Wqkv
    C = Bn @ Wqkv + bqkv

    seq = np.arange(NSEQ)
    g = seq // 64
    j = seq % 64
    p_t = np.zeros((PAD, D), np.int64)
    for t in range(D):
        p_t[:NSEQ, t] = perm[4 * g + t, j]
    valid = np.arange(PAD) < NSEQ
    # coefficient-side slots: q = p*13 + k
    q_of = np.arange(PAD)
    cs_p, cs_k = q_of // NBLK, q_of % NBLK
    out = {}

    # logits Gram tables: gvh[p, br*208 + kind*52 + k*4 + t] for q=p*13+k
    gv = np.zeros((128, 416), np.float32)
    for br in range(2):
        Aq, Ak = A[:, sl(3 * br)], A[:, sl(3 * br + 1)]
        Cq, Ck = C[:, sl(3 * br)], C[:, sl(3 * br + 1)]
        Gt = (Aq @ Ak.T, Aq @ Ck.T, Cq @ Ak.T, Cq @ Ck.T)
        for kind in range(4):
            for t in range(D):
                v = np.zeros(PAD, np.float32)
                v[valid] = Gt[kind][p_t[valid, 0], p_t[valid, t]]
                gv[cs_p, br * 208 + kind * 52 + cs_k * 4 + t] = v
    out['gvh'] = gv

    # one-hot scatter-transpose masks [128, 3328], col = b*1664 + q
    oht = {}
    for t in range(D):
        oh = np.zeros((N_COND, PAD), np.float32)
        for s in range(NSEQ):
            oh[p_t[s, t], s] += 1.0
        ohd = np.concatenate([oh, oh], 0)
        oht[t] = np.tile(ohd, (1, 2)).astype(BF)
    for t in range(D):
        out[f'oht{t}'] = oht[t]
    oh0 = np.zeros((N_COND, PAD), np.float32)
    for s in range(NSEQ):
        oh0[p_t[s, 0], s] += 1.0
    out['oht0t'] = np.tile(oh0, (1, 2)).astype(BF)  # [64, 3328]

    Av, Cv = {}, {}
    for br in range(2):
        WV, bV = Wqkv[:, sl(3 * br + 2)], bqkv[sl(3 * br + 2)]
        Av[br] = Wn @ WV
        Cv[br] = Bn @ WV + bV
    t1w = np.concatenate([Av[0], Cv[0]], 0).astype(BF)
    t1e = np.concatenate([Av[1], Cv[1]], 0).astype(BF)
    t2 = np.concatenate([Wn, Bn], 0).astype(BF)

    # gathered row-mean tables, coefficient slots [p, b*52 + k*4 + t]
    gm = {}
    for br in range(2):
        ta = np.zeros((128, 104), np.float32)
        tcn = np.zeros((128, 104), np.float32)
        rmA, rmC = Av[br].mean(1), Cv[br].mean(1)
        for t in range(D):
            va = np.zeros(PAD, np.float32)
            vc = np.zeros(PAD, np.float32)
            va[valid] = rmA[p_t[valid, t]]
            vc[valid] = rmC[p_t[valid, t]]
            for b in range(2):
                ta[cs_p, b * 52 + cs_k * 4 + t] = va
                tcn[cs_p, b * 52 + cs_k * 4 + t] = vc
        gm[(br, 'a')] = ta
        gm[(br, 'c')] = tcn
    g0 = np.zeros((128, 26), np.float32)
    gB = np.zeros((128, 26), np.float32)
    rmW, rmB = Wn.mean(1), Bn.mean(1)
    v0 = np.zeros(PAD, np.float32)
    vB = np.zeros(PAD, np.float32)
    v0[valid] = rmW[p_t[valid, 0]]
    vB[valid] = rmB[p_t[valid, 0]]
    for b in range(2):
        g0[cs_p, b * 13 + cs_k] = v0
        gB[cs_p, b * 13 + cs_k] = vB

    Wowg = inp['gamma_w'].astype(np.float32) * inp['Wow'][:, 0].astype(np.float32)
    wowgr = np.tile(Wowg[None, :], (128, NBLK)).astype(BF)

    # forest mask in transposed slots: m01[p', kk*100+f] = M01[kk*128+p', f]
    swr = inp['swr'].astype(np.int64)
    M01 = np.zeros((PAD, N_FOREST), np.float32)
    for f in range(N_FOREST):
        r = swr[f]
        s = (r % 25) * 64 + (r // 25)
        M01[s, f] = 1.0
    m01 = M01.reshape(NBLK, 128, N_FOREST).transpose(1, 0, 2).reshape(128, NBLK * N_FOREST).astype(BF)

    tailc = np.zeros((128, 676), np.float32)
    tailc[0:1, 420:548] = 1.0
    tailc[0:1, 548:676] = -1.0
    tailc[:, 0:128] = inp['gamma_E'].astype(np.float32)[:, None] * inp['WoE'].astype(np.float32)
    tailc[:, 128:256] = inp['g1'].astype(np.float32)[:, None] * inp['W1'].astype(np.float32)
    W2p = inp['g2'].astype(np.float32)[:, None] * inp['W2'].astype(np.float32)
    tailc[:, 256:272] = np.concatenate([W2p, np.zeros((H, 6), np.float32)], 1)
    tailc[:, 272:273] = (inp['be1'] @ inp['W1'] + inp['b1'])[:, None].astype(np.float32)
    tailc[:, 273:274] = 1.0
    tailc[:, 274:275] = EPS
    tailc[:, 275:276] = float(inp['beta_w'] @ inp['Wow'][:, 0] + inp['bow'][0])
    b2p = (inp['be2'] @ inp['W2'] + inp['b2']).astype(np.float32) / N_FOREST
    tailc[0:16, 276:277] = np.concatenate([b2p, np.zeros(6, np.float32)])[:, None]
    tailc[0:1, 277:405] = (inp['beta_E'] @ inp['WoE'] + inp['boE'])[None, :].astype(np.float32)

    # fp32 pack 2
    cst2 = np.zeros((128, C2_W), np.float32)
    cst2[:, C2_GMAW:C2_GMAW + 104] = gm[(0, 'a')]
    cst2[:, C2_GMCW:C2_GMCW + 104] = gm[(0, 'c')]
    cst2[:, C2_GMAE:C2_GMAE + 104] = gm[(1, 'a')]
    cst2[:, C2_GMCE:C2_GMCE + 104] = gm[(1, 'c')]
    cst2[:, C2_GM0:C2_GM0 + 26] = g0
    cst2[:, C2_GMB:C2_GMB + 26] = gB
    cst2[:, C2_TAIL:C2_TAIL + 676] = tailc
    out['cst2'] = cst2

    # bf16 pack
    cbf = np.zeros((128, CB_W), BF)
    cbf[:, CB_T1W:CB_T1W + 128] = t1w
    cbf[:, CB_T1E:CB_T1E + 128] = t1e
    cbf[:, CB_T2:CB_T2 + 128] = t2
    cbf[:, CB_M01:CB_M01 + NBLK * 100] = m01
    cbf[:, CB_WOW:CB_WOW + NBLK * 128] = wowgr
    cbf[0:1, CB_NEG:CB_NEG + 128] = -1.0
    cbf[32:33, CB_NEG:CB_NEG + 128] = -1.0
    WoEg_t = tailc[:, 0:128]
    W1p_t = tailc[:, 128:256]
    W2B_t = tailc[:, 256:272] / N_FOREST
    cbf[:, CB_WOE:CB_WOE + 128] = WoEg_t
    cbf[:, CB_W1B:CB_W1B + 128] = W1p_t
    cbf[:, CB_W2B:CB_W2B + 16] = W2B_t
    cbf[:, CB_ONEHC:CB_ONEHC + 1] = 1.0 / H
    cbf[0:1, CB_ONE1:CB_ONE1 + 128] = 1.0
    cbf[0:1, CB_BOE2:CB_BOE2 + 128] = tailc[0:1, 277:405]
    cbf[0:1, CB_W1SN:CB_W1SN + 128] = -W1p_t.sum(0)[None, :]
    cbf[0:1, CB_W2SN:CB_W2SN + 16] = -W2B_t.sum(0)[None, :]
    out['cbf'] = cbf
    return out


def _host_x(inp, bs, gvh):
    x = inp['x'].astype(np.float32)
    q_of = np.arange(NSEQ)
    cs_p, cs_k = q_of // NBLK, q_of % NBLK
    g = q_of // 64
    p1t = np.zeros((128, P1T_W), np.float32)
    p1t[:, P1T_GVH:P1T_GVH + 416] = gvh
    for bi, b in enumerate(bs):
        for t in range(D):
            p1t[cs_p, P1T_XD + bi * 52 + cs_k * 4 + t] = x[b, 4 * g + t]
        p1t[cs_p, P1T_XD + 104 + bi * 13 + cs_k] = x[b, 4 * g]
    return {'p1t': p1t}


_H_SHAPES = {
    'p1t': ((128, P1T_W), F32),
    'cst2': ((128, C2_W), F32),
    'oht0': ((128, W2), BF16), 'oht1': ((128, W2), BF16),
    'oht2': ((128, W2), BF16), 'oht3': ((128, W2), BF16),
    'oht0t': ((64, W2), BF16),
    'cbf': ((128, CB_W), BF16),
}


def _vw(ap, off, dims):
    return bass.AP(tensor=ap.tensor, offset=ap.offset + off,
                   ap=[list(ap.ap[0])] + [[s, c] for (s, c) in dims])


def _vw1(ap):
    return bass.AP(tensor=ap.tensor, offset=ap.offset,
                   ap=[[ap.ap[0][0], 1]] + [list(d) for d in ap.ap[1:]])


def _dram_ap(handle, off, dims):
    return bass.AP(tensor=handle.tensor, offset=handle.offset + off,
                   ap=[[s, c] for (s, c) in dims])


DEBUG = False


def _build_nc():
    nc = bacc_mod.Bacc()
    dram = {k: nc.declare_dram_parameter(k, list(sh), dt, isOutput=False)
            for k, (sh, dt) in _H_SHAPES.items()}
    out_d = nc.declare_dram_parameter('out', [2, 16], F32, isOutput=True)
    if DEBUG:
        dbg = {
            'd_mbf': nc.declare_dram_parameter('d_mbf', [128, 208], BF16, isOutput=True),
            'd_ct00': nc.declare_dram_parameter('d_ct00', [128, W2], BF16, isOutput=True),
            'd_ct11': nc.declare_dram_parameter('d_ct11', [128, W2], BF16, isOutput=True),
            'd_ss0': nc.declare_dram_parameter('d_ss0', [128, W2], BF16, isOutput=True),
            'd_xs0': nc.declare_dram_parameter('d_xs0', [128, W2], BF16, isOutput=True),
            'd_mur': nc.declare_dram_parameter('d_mur', [2, W2], BF16, isOutput=True),
            'd_wr2': nc.declare_dram_parameter('d_wr2', [128, W2], BF16, isOutput=True),
            'd_muf': nc.declare_dram_parameter('d_muf', [128, 52], F32, isOutput=True),
        }
    rowt = nc.dram_tensor('rowt', [N_ROWS, W2], BF16)
    rbase = rowt[0, 0:1]

    with TileContext(nc) as tc:
        with (
            tc.tile_pool(name='const', bufs=1) as cp,
            tc.tile_pool(name='work', bufs=1) as wk,
            tc.tile_pool(name='cpT', bufs=5) as cpT,
            tc.tile_pool(name='scr2', bufs=3) as scr2,
            tc.tile_pool(name='px', bufs=2, space='PSUM') as px,
            tc.tile_pool(name='pt', bufs=2, space='PSUM') as pt_pool,
            tc.tile_pool(name='pz', bufs=2, space='PSUM') as pz,
        ):
            tl = {}
            # critical P1/P2 tables first on sync with the full DMA bus; the
            # big mask loads are gated behind p1t arrival via a tiny WAW
            # write into each destination tile so they can't contend
            for k, q in (('p1t', nc.sync), ('cst2', nc.sync)):
                sh, dt = _H_SHAPES[k]
                t = cp.tile(list(sh), dt, name=f'c_{k}')
                q.dma_start(out=t[...], in_=dram[k][...])
                tl[k] = t
            touch = cp.tile([128, 1], F32, tag='touch')

            def _touch(src, eng=None):
                (eng or nc.vector).tensor_copy(touch[0:src.ap[0][1], 0:1], src)

            wrhs2 = wk.tile([128, W2], BF16, tag='wrhs2')
            for k, q in (('oht0', nc.scalar), ('oht2', nc.gpsimd),
                         ('oht1', nc.scalar), ('oht3', nc.gpsimd),
                         ('oht0t', nc.scalar), ('cbf', nc.gpsimd)):
                sh, dt = _H_SHAPES[k]
                t = cp.tile(list(sh), dt, name=f'c_{k}')
                nc.gpsimd.tensor_copy(t[0:1, 0:1], tl['p1t'][0:1, 0:1])
                q.dma_start(out=t[...], in_=dram[k][...])
                tl[k] = t
            nc.gpsimd.tensor_copy(wrhs2[64:65, 0:1], tl['p1t'][0:1, 0:1])
            nc.scalar.dma_start(out=wrhs2[64:128, :], in_=dram['oht0t'][...])

            cst2 = tl['cst2']
            tailc = cst2[:, C2_TAIL:C2_TAIL + 676]
            WoEg = tailc[:, 0:128]
            W1p = tailc[:, 128:256]
            W2pc = tailc[:, 256:272]
            b1p = tailc[:, 272:273]
            ones = tailc[:, 273:274]
            eps_sb = tailc[:, 274:275]
            bow2 = tailc[:, 275:276]
            b2p = tailc[0:16, 276:277]
            boE2 = tailc[0:1, 277:405]
            onesrow = tailc[0:1, 420:548]
            cbf = tl['cbf']
            negrow = cbf[0:1, CB_NEG:CB_NEG + 128]
            oht = {t: tl[f'oht{t}'][:, :] for t in range(D)}

            _touch(tl['p1t'][:, 0:1])
            _touch(cst2[:, 0:1])

            # ---------------- P1: softmax per batch -> m/a ----------------
            m_all = wk.tile([128, 208], F32, tag='m_all')
            a_all = wk.tile([128, 208], F32, tag='a_all')
            # bf16, export layout col = br*104 + t*26 + b*13 + k
            m_bf = wk.tile([128, 208], BF16, tag='m_bf')
            a_bf = wk.tile([128, 208], BF16, tag='a_bf')
            x0_bf = wk.tile([128, 26], BF16, tag='x0_bf')
            xt_rows = tl['p1t'][:, P1T_XD:P1T_XD + 104]
            x0_rows = tl['p1t'][:, P1T_XD + 104:P1T_XD + 130]
            nc.scalar.copy(x0_bf[:, :], x0_rows)
            gvv = lambda kind: _vw(tl['p1t'][:, :], P1T_GVH + kind * 52,
                                   [(208, 2), (4, NBLK), (1, 4)])
            for b in range(2):
                xtv = _vw(xt_rows, b * 52, [(0, 2), (4, NBLK), (1, 4)])
                x0v = _vw(x0_rows, b * NBLK, [(0, 2), (1, NBLK), (0, 4)])
                t1 = wk.tile([128, 104], F32, name=f't1_{b}')
                t2 = wk.tile([128, 104], F32, name=f't2_{b}')
                nc.vector.tensor_mul(t1[:, :], gvv(0), xtv)
                nc.gpsimd.tensor_mul(t2[:, :], gvv(2), xtv)
                nc.vector.tensor_add(t1[:, :], t1[:, :], gvv(1))
                nc.gpsimd.tensor_add(t2[:, :], t2[:, :], gvv(3))
                nc.vector.tensor_mul(t1[:, :], t1[:, :], x0v)
                nc.vector.tensor_add(t1[:, :], t1[:, :], t2[:, :])
                e = wk.tile([128, 104], F32, name=f'e_{b}')
                nc.scalar.activation(e[:, :], t1[:, :], AF.Exp, bias=0.0, scale=S128)
                esum = wk.tile([128, 26], F32, name=f'esum_{b}')
                nc.vector.tensor_reduce(esum[:, :], _vw(e[:, :], 0, [(4, 26), (1, 4)]),
                                        mybir.AxisListType.X, OP.add)
                nc.vector.reciprocal(esum[:, :], esum[:, :])
                asl = a_all[:, b * 104:(b + 1) * 104]
                msl = m_all[:, b * 104:(b + 1) * 104]
                nc.vector.tensor_mul(asl, e[:, :], _vw(esum[:, :], 0, [(1, 26), (0, 4)]))
                nc.vector.tensor_mul(msl, asl, xtv)
                for src, dst in ((a_all, a_bf), (m_all, m_bf)):
                    nc.scalar.copy(
                        _vw(dst[:, :], b * 13, [(104, 2), (26, 4), (1, 13)]),
                        _vw(src[:, :], b * 104, [(52, 2), (1, 4), (4, 13)]))

            # ---------------- P3/P4: export rows + broadcast imports -------
            # one export DMA per (br, kind): 4 rows each; dst rows br*8+t(+4)
            # src cols t*26+b*13+k are contiguous (t,b,k) t-major, matching
            # dst dims (p, tb, k): consecutive t rows are W2 apart and b
            # halves PAD apart, so (t,b) merges to stride PAD count 8
            XSRC = [(1, 104)]
            XDST = [(NBLK, 128), (PAD, 8), (1, NBLK)]
            for br in range(2):
                for src, r0 in ((m_bf, ROW_M(br, 0)), (a_bf, ROW_A(br, 0))):
                    nc.sync.dma_start(
                        out=_dram_ap(rbase, r0 * W2, XDST),
                        in_=_vw(src[:, :], br * 104, XSRC))
            # x0 export: dst row 16, dims (p, b, k)
            nc.sync.dma_start(
                out=_dram_ap(rbase, ROW_X0 * W2, [(NBLK, 128), (PAD, 2), (1, NBLK)]),
                in_=_vw(x0_bf[:, :], 0, [(NBLK, 2), (1, NBLK)]))

            # ct imports: one DMA per (br,t), m half -> parts 0:64, a -> 64:128
            coefT = {}
            iq = {0: nc.scalar, 1: nc.gpsimd}
            for br in range(2):
                for t in range(D):
                    ct = cpT.tile([128, W2], BF16, name='ct')
                    iq[br].dma_start(
                        out=ct[0:64, :],
                        in_=_dram_ap(rbase, ROW_M(br, t) * W2, [(0, 64), (1, W2)]))
                    iq[br].dma_start(
                        out=ct[64:128, :],
                        in_=_dram_ap(rbase, ROW_A(br, t) * W2, [(0, 64), (1, W2)]))
                    coefT[(br, t)] = ct

            # ---------------- P2: mu via gathered row-means ----------------
            mu = wk.tile([128, 52], F32, tag='mu')       # col = br*26 + b*13 + k
            mu_bf = wk.tile([128, 52], BF16, tag='mu_bf')
            for br, (ga, gc) in enumerate(((C2_GMAW, C2_GMCW), (C2_GMAE, C2_GMCE))):
                pm = wk.tile([128, 104], F32, name=f'pm_{br}')
                pa = wk.tile([128, 104], F32, name=f'pa_{br}')
                mview = _vw(m_all[:, :], br * 52, [(104, 2), (1, 52)])
                aview = _vw(a_all[:, :], br * 52, [(104, 2), (1, 52)])
                nc.vector.tensor_mul(pm[:, :], mview, cst2[:, ga:ga + 104])
                nc.gpsimd.tensor_mul(pa[:, :], aview, cst2[:, gc:gc + 104])
                nc.vector.tensor_add(pm[:, :], pm[:, :], pa[:, :])
                musl = mu[:, br * 26:(br + 1) * 26]
                nc.vector.tensor_reduce(musl, _vw(pm[:, :], 0, [(52, 2), (4, 13), (1, 4)]),
                                        mybir.AxisListType.X, OP.add)
                x0g = wk.tile([128, 26], F32, name=f'x0g_{br}')
                nc.gpsimd.tensor_mul(x0g[:, :], x0_rows, cst2[:, C2_GM0:C2_GM0 + 26])
                nc.vector.tensor_add(musl, musl, x0g[:, :])
                nc.vector.tensor_add(musl, musl, cst2[:, C2_GMB:C2_GMB + 26])
                nc.scalar.copy(mu_bf[:, br * 26:(br + 1) * 26], musl)

            # mu export: both rows in one DMA; dims (p, br*b, k) — (br,b)
            # merges since mu rows are W2 apart
            nc.sync.dma_start(
                out=_dram_ap(rbase, ROW_MU(0) * W2,
                             [(NBLK, 128), (PAD, 4), (1, NBLK)]),
                in_=_vw(mu_bf[:, :], 0, [(NBLK, 4), (1, NBLK)]))

            x0bc_t = wk.tile([64, W2], BF16, tag='x0bc_t')
            x0bc = x0bc_t[:, :]
            nc.sync.dma_start(out=x0bc,
                              in_=_dram_ap(rbase, ROW_X0 * W2, [(0, 64), (1, W2)]))
            murows = wk.tile([33, W2], BF16, tag='murows')
            murow = {0: murows[0:1, :], 1: murows[32:33, :]}
            for br in range(2):
                nc.sync.dma_start(
                    out=murow[br],
                    in_=_dram_ap(rbase, ROW_MU(br) * W2, [(0, 1), (1, W2)]))

            # ---------------- P5: SS1 scatter build ------------------------
            SS1 = {}
            sstmp = {}
            for br in range(2):
                ss = wk.tile([128, W2], BF16, name=f'ss1_{br}')
                tmpa = wk.tile([128, W2], BF16, name=f'sstmpa_{br}')
                tmpb = wk.tile([128, W2], BF16, name=f'sstmpb_{br}')
                SS1[br] = ss
                sstmp[br] = (tmpa, tmpb)
            _touch(x0bc[0:1, 0:1])
            nc.gpsimd.tensor_mul(wrhs2[0:64, :], tl['oht0t'][:, :], x0bc)
            for br in range(2):
                eng = nc.vector
                ss = SS1[br]
                tmpa, tmpb = sstmp[br]
                dsts = (ss, tmpa, tmpb, tmpa)
                for t in range(D):
                    ct = coefT[(br, t)]
                    _touch(ct[:, 0:1], eng)
                    if DEBUG and (br, t) == (0, 0):
                        nc.sync.dma_start(out=dbg['d_ct00'][...], in_=ct[...])
                    if DEBUG and (br, t) == (1, 1):
                        nc.sync.dma_start(out=dbg['d_ct11'][...], in_=ct[...])
                    eng.tensor_mul(dsts[t][:, :], oht[t][:, :], ct[:, :])
                    if t == 1:
                        eng.tensor_add(ss[:, :], ss[:, :], tmpa[:, :])
                eng.tensor_add(ss[:, :], ss[:, :], tmpb[:, :])
                eng.tensor_add(ss[:, :], ss[:, :], tmpa[:, :])
            for br in range(2):
                _touch(murow[br][0:1, 0:1])

            # ---------------- P6/P7: x_inT matmuls + copy out --------------
            t1t = {0: cbf[:, CB_T1W:CB_T1W + 128], 1: cbf[:, CB_T1E:CB_T1E + 128]}
            t2c = cbf[:, CB_T2:CB_T2 + 128]
            xs = {}
            for br in range(2):
                x_s = wk.tile([128, W2], BF16, name=f'xs_{br}')
                xs[br] = x_s
                for ci, (c0, c1) in enumerate(CHUNKS):
                    pch = px.tile([128, CHUNK], F32, name='pxc')[:, 0:c1 - c0]
                    nc.tensor.matmul(pch[:, :], t1t[br][:, :], SS1[br][:, c0:c1],
                                     start=True, stop=False)
                    nc.tensor.matmul(pch[:, :], t2c, wrhs2[:, c0:c1],
                                     start=False, stop=False)
                    nc.tensor.matmul(pch[:, :],
                                     cbf[32 * br:32 * br + 1, CB_NEG:CB_NEG + 128],
                                     murow[br][0:1, c0:c1], start=False, stop=True)
                    nc.scalar.copy(x_s[:, c0:c1], pch[:, :])

            # ---------------- P8: DMA transpose to seq-slot layout ---------
            qeng = (nc.sync, nc.scalar)
            xin = {}
            for br in range(2):
                for b in range(2):
                    xt_ = wk.tile([128, NBLK, 128], BF16, name=f'xin_{br}_{b}')
                    qeng[br].dma_start(out=xt_[:, :, :],
                                       in_=xs[br][:, b * PAD:(b + 1) * PAD],
                                       transpose=True)
                    xin[(br, b)] = xt_

            # ---------------- P9: stats ------------------------------------
            veng = (nc.vector, nc.gpsimd)
            ssq = wk.tile([128, 52], F32, tag='ssq')      # col = br*26 + b*13 + kk
            wraw = wk.tile([128, 26], F32, tag='wraw')
            for br in range(2):
                for b in range(2):
                    xt_ = xin[(br, b)]
                    _touch(xt_[:, 0, 0:1], veng[br])
                    sq = scr2.tile([128, PAD], BF16, name='sqt')
                    nc.scalar.square(sq[:, :], _vw(xt_[:, :, :], 0, [(1, PAD)]))
                    nc.vector.tensor_reduce(
                        ssq[:, br * 26 + b * 13: br * 26 + (b + 1) * 13],
                        _vw(sq[:, :], 0, [(128, NBLK), (1, 128)]),
                        mybir.AxisListType.X, OP.add)
            for b in range(2):
                wx = scr2.tile([128, PAD], BF16, name='sqt')
                nc.gpsimd.tensor_mul(wx[:, :], _vw(xin[(0, b)][:, :, :], 0, [(1, PAD)]),
                                     cbf[:, CB_WOW:CB_WOW + NBLK * 128])
                nc.vector.tensor_reduce(wraw[:, b * 13:(b + 1) * 13],
                                        _vw(wx[:, :], 0, [(128, NBLK), (1, 128)]),
                                        mybir.AxisListType.X, OP.add)
            stdv = wk.tile([128, 52], F32, tag='stdv')
            nc.scalar.activation(stdv[:, :], ssq[:, :], AF.Sqrt,
                                 bias=eps_sb, scale=1.0 / H)
            rstd = wk.tile([128, 52], F32, tag='rstd')
            nc.vector.reciprocal(rstd[:, :], stdv[:, :])
            o2 = wk.tile([128, 26], F32, tag='o2')
            nc.vector.tensor_mul(o2[:, :], wraw[:, :], rstd[:, 0:26])
            expw = wk.tile([128, 26], F32, tag='expw')
            nc.scalar.activation(expw[:, :], o2[:, :], AF.Exp, bias=bow2, scale=1.0)
            er = wk.tile([128, 26], F32, tag='er')
            nc.vector.tensor_mul(er[:, :], expw[:, :], rstd[:, 26:52])
            std_bf = wk.tile([128, 26], BF16, tag='std_bf')
            nc.scalar.copy(std_bf[:, :], stdv[:, 26:52])

            # ---------------- P10: forest ----------------------------------
            # main_c = sum_seq er*E_c*M01 via 13 accumulating matmuls; the
            # forest softmax denominator reuses M01er: z = sum std_E*M01er
            m01c = cbf[:, CB_M01:CB_M01 + NBLK * 100]
            psC, psZ = {}, {}
            for b in range(2):
                m01er = wk.tile([128, NBLK * 100], BF16, name=f'm01er_{b}')
                nc.vector.tensor_mul(
                    m01er[:, :], m01c,
                    _vw(er[:, :], b * NBLK, [(1, NBLK), (0, 100)]))
                pc = pt_pool.tile([128, 200], F32, name='tailps')[:, 0:100]
                pzt = pz.tile([16, 200], F32, name='rowps')[0:1, 0:100]
                for k in range(NBLK):
                    nc.tensor.matmul(pc, xin[(1, b)][:, k, :],
                                     m01er[:, k * 100:(k + 1) * 100],
                                     start=(k == 0), stop=(k == NBLK - 1))
                    nc.tensor.matmul(pzt, std_bf[:, b * 13 + k: b * 13 + k + 1],
                                     m01er[:, k * 100:(k + 1) * 100],
                                     start=(k == 0), stop=(k == NBLK - 1),
                                     skip_group_check=True)
                psC[b], psZ[b] = pc, pzt

            # ---------------- P11: tail (batch-pair fused) -----------------
            main_s = wk.tile([128, 200], F32, tag='main_s')
            z_s = wk.tile([1, 200], F32, tag='z_s')
            for b in range(2):
                nc.scalar.copy(main_s[:, b * 100:(b + 1) * 100], psC[b])
                nc.vector.tensor_copy(z_s[0:1, b * 100:(b + 1) * 100], psZ[b])
            pt = pt_pool.tile([128, 200], F32, name='tailps')
            nc.tensor.matmul(pt[:, :], WoEg, main_s[:, :], start=True, stop=False)
            nc.tensor.matmul(pt[:, :], boE2, z_s[0:1, :], start=False, stop=True,
                             skip_group_check=True)
            rz = wk.tile([1, 200], F32, tag='rz')
            nc.vector.reciprocal(rz[:, :], z_s[:, :])
            rzb = px.tile([128, CHUNK], F32, name='pxc')[:, 0:200]
            nc.tensor.matmul(rzb, onesrow, rz[0:1, :], start=True, stop=True)
            ft_s = wk.tile([128, 200], F32, tag='ft_s')
            nc.scalar.copy(ft_s[:, :], pt[:, :])
            F_s = wk.tile([128, 200], F32, tag='F_s')
            nc.vector.tensor_mul(F_s[:, :], ft_s[:, :], rzb)

            def ln_cols(V, nm):
                cs1 = pz.tile([16, 200], F32, name='rowps')[0:1, :]
                cs2 = pz.tile([16, 200], F32, name='rowps')[0:1, :]
                sqv = wk.tile([128, 200], F32, name=f'sqv_{nm}')
                nc.gpsimd.tensor_mul(sqv[:, :], V, V)
                nc.tensor.matmul(cs1, ones, V, start=True, stop=True)
                nc.tensor.matmul(cs2, ones, sqv[:, :], start=True, stop=True)
                strow = wk.tile([1, 400], F32, name=f'st_{nm}')
                tmp = wk.tile([1, 200], F32, name=f'tmp_{nm}')
                nc.vector.tensor_scalar_mul(strow[0:1, 0:200], cs1, 1.0 / H)
                nc.vector.tensor_scalar_mul(strow[0:1, 200:400], cs2, 1.0 / H)
                nc.vector.tensor_mul(tmp[:, :], strow[0:1, 0:200], strow[0:1, 0:200])
                nc.vector.tensor_sub(strow[0:1, 200:400], strow[0:1, 200:400], tmp[:, :])
                nc.scalar.activation(strow[0:1, 200:400], strow[0:1, 200:400],
                                     AF.Sqrt, bias=_vw1(eps_sb), scale=1.0)
                nc.vector.reciprocal(strow[0:1, 200:400], strow[0:1, 200:400])
                # strow[0:200] <- mu*rstd
                nc.vector.tensor_mul(strow[0:1, 0:200], strow[0:1, 0:200],
                                     strow[0:1, 200:400])
                mb = px.tile([128, CHUNK], F32, name='pxc')[:, 0:400]
                nc.tensor.matmul(mb, onesrow, strow[0:1, :], start=True, stop=True)
                LN = wk.tile([128, 200], F32, name=f'ln_{nm}')
                nc.vector.tensor_mul(LN[:, :], V, mb[:, 200:400])
                nc.vector.tensor_sub(LN[:, :], LN[:, :], mb[:, 0:200])
                return LN

            LN1 = ln_cols(F_s[:, :], 'l1')
            pt2 = pt_pool.tile([128, 200], F32, name='tailps')
            nc.tensor.matmul(pt2[:, :], W1p, LN1[:, :], start=True, stop=True)
            h1 = wk.tile([128, 200], F32, tag='h1')
            nc.scalar.activation(h1[:, :], pt2[:, :], AF.Relu, bias=b1p, scale=1.0)
            LN2 = ln_cols(h1[:, :], 'l2')
            po = pz.tile([16, 200], F32, name='rowps')
            nc.tensor.matmul(po[0:16, :], W2pc, LN2[:, :], start=True, stop=True)
            ob = wk.tile([16, 200], F32, tag='ob')
            nc.scalar.activation(ob[:, :], po[0:16, :], AF.Identity, bias=b2p, scale=1.0)
            ored = wk.tile([16, 2], F32, tag='ored')
            nc.vector.tensor_reduce(ored[:, :], _vw(ob[:, :], 0, [(100, 2), (1, 100)]),
                                    mybir.AxisListType.X, OP.add)
            ofin = wk.tile([16, 2], F32, tag='ofin')
            nc.vector.tensor_scalar_mul(ofin[:, :], ored[:, :], 1.0 / N_FOREST)
            for b in range(2):
                nc.sync.dma_start(out=out_d[b, :], in_=ofin[:, b:b + 1])
            if DEBUG:
                nc.sync.dma_start(out=dbg['d_mbf'][...], in_=m_bf[...])
                nc.sync.dma_start(out=dbg['d_ss0'][...], in_=SS1[0][...])
                nc.sync.dma_start(out=dbg['d_muf'][...], in_=mu[...])
                nc.sync.dma_start(out=dbg['d_xs0'][...], in_=xs[0][...])
                nc.sync.dma_start(out=dbg['d_mur'][0, :], in_=murow[0])
                nc.sync.dma_start(out=dbg['d_mur'][1, :], in_=murow[1])
                nc.sync.dma_start(out=dbg['d_wr2'][...], in_=wrhs2[...])
    nc.finalize()
    return nc


_NC_CACHE = {}


def kernel(**inputs):
    inp = {k: np.asarray(v) for k, v in inputs.items()}
    H_ = _host_precompute(inp)
    if 'nc' not in _NC_CACHE:
        _NC_CACHE['nc'] = _build_nc()
    nc = _NC_CACHE['nc']
    gvh = H_.pop('gvh')
    in_maps = []
    for c in range(NCORES):
        m = {k: np.ascontiguousarray(H_[k]) for k in H_}
        m.update({k: np.ascontiguousarray(v)
                  for k, v in _host_x(inp, (2 * c, 2 * c + 1), gvh).items()})
        in_maps.append(m)
    res = run_bass_kernel_spmd(nc, in_maps, list(range(NCORES)))
    out = np.zeros((B, N_CLASS), np.float32)
    for c in range(NCORES):
        out[2 * c:2 * c + 2] = res.results[c]['out'][:, :N_CLASS]
    return out


# revision 46
# speedup vs baseline: 1.1059x; 1.0336x over previous
"""DOFENTransformer Trainium2 kernel, v2.

Data-parallel: 16 batches / 8 cores = one batch-PAIR per core. The
per-token attention-output rows (x_in = attn_out + residual) are built on
the TensorEngine instead of per-block vector FMA chains:

  x_inT[h, q] = T1^T @ SS1 + T2^T @ rhs2 - ones^T @ mu_row
    SS1[c | c+64, q] = one-hot scatter of softmax coefficients (m, a),
    built with 7 wide tensor_tensor ops per branch from host one-hot masks
    and DMA-broadcast coefficient rows; rhs2 carries the x0/residual
    terms. mu (row means) comes from host-gathered table means, so x_in
    arrives mean-centered and LayerNorm needs only a sum of squares.

Column q of x_inT holds rODT seq q. Coefficient-side tiles use slot
(p, k) = (q // 13, q % 13); x_inT is DMA-XBAR-transposed to seq-slot
layout (p', kk) = (q % 128, q // 128) for the forest contraction (PE) and
stats (strided reduces). Host tables are gathered per-slot so the two
layouts never mix on device. bf16 for all wide ops; fp32 stats + tail.
"""
import sys

for p in ('/opt/trn_rl_repo', '/root/.axon_site/_ro/trn_rl_repo'):
    if p not in sys.path:
        sys.path.insert(0, p)

import numpy as np
import ml_dtypes
import concourse.bass as bass
import concourse.bacc as bacc_mod
from concourse import mybir
from concourse.tile import TileContext
from concourse.bass_utils import run_bass_kernel_spmd

B, N_COL, N_COND, D, H = 16, 100, 64, 4, 128
N_FOREST, N_CLASS = 100, 10
NSEQ, NBLK, PAD = 1600, 13, 1664
W2 = 2 * PAD  # batch-pair width 3328
EPS = 1e-5
S128 = float(np.sqrt(128.0))
F32 = mybir.dt.float32
BF16 = mybir.dt.bfloat16
AF = mybir.ActivationFunctionType
OP = mybir.AluOpType
NCORES = 8
BF = ml_dtypes.bfloat16

CHUNK = 512
CHUNKS = [(c, min(c + CHUNK, W2)) for c in range(0, W2, CHUNK)]

ROW_M = lambda br, t: br * 4 + t
ROW_A = lambda br, t: 8 + br * 4 + t
ROW_X0 = 16
ROW_MU = lambda br: 17 + br
N_ROWS = 19

TB_WOE, TB_W1B, TB_W2B = 0, 128, 256
TB_ONEHC, TB_ONE1, TB_BOE2, TB_W1SN, TB_W2SN = 272, 273, 401, 529, 657


def _host_precompute(inp):
    sl = lambda i: slice(i * H, (i + 1) * H)
    Wn = inp['W_num'].reshape(N_COND, H).astype(np.float32)
    Bn = inp['b_num'].reshape(N_COND, H).astype(np.float32)
    Wqkv, bqkv = inp['Wqkv'].astype(np.float32), inp['bqkv'].astype(np.float32)
    perm = inp['perm'].astype(np.int64)
    A = Wn @ Wqkv
    C = Bn @ Wqkv + bqkv

    seq = np.arange(NSEQ)
    g = seq // 64
    j = seq % 64
    p_t = np.zeros((PAD, D), np.int64)
    for t in range(D):
        p_t[:NSEQ, t] = perm[4 * g + t, j]
    valid = np.arange(PAD) < NSEQ
    # coefficient-side slots: q = p*13 + k
    q_of = np.arange(PAD)
    cs_p, cs_k = q_of // NBLK, q_of % NBLK
    out = {}

    # logits Gram tables: gvh[p, br*208 + kind*52 + k*4 + t] for q=p*13+k
    gv = np.zeros((128, 416), np.float32)
    for br in range(2):
        Aq, Ak = A[:, sl(3 * br)], A[:, sl(3 * br + 1)]
        Cq, Ck = C[:, sl(3 * br)], C[:, sl(3 * br + 1)]
        Gt = (Aq @ Ak.T, Aq @ Ck.T, Cq @ Ak.T, Cq @ Ck.T)
        for kind in range(4):
            for t in range(D):
                v = np.zeros(PAD, np.float32)
                v[valid] = Gt[kind][p_t[valid, 0], p_t[valid, t]]
                gv[cs_p, br * 208 + kind * 52 + cs_k * 4 + t] = v
    out['gvh'] = gv

    # one-hot scatter-transpose masks [128, 3328], col = b*1664 + q
    for t in range(D):
        oh = np.zeros((N_COND, PAD), np.float32)
        for s in range(NSEQ):
            oh[p_t[s, t], s] += 1.0
        ohd = np.concatenate([oh, oh], 0)
        out[f'oht{t}'] = np.tile(ohd, (1, 2)).astype(BF)
    oh0 = np.zeros((N_COND, PAD), np.float32)
    for s in range(NSEQ):
        oh0[p_t[s, 0], s] += 1.0
    out['oht0t'] = np.tile(oh0, (1, 2)).astype(BF)  # [64, 3328]

    Av, Cv = {}, {}
    for br in range(2):
        WV, bV = Wqkv[:, sl(3 * br + 2)], bqkv[sl(3 * br + 2)]
        Av[br] = Wn @ WV
        Cv[br] = Bn @ WV + bV
    out['t1w'] = np.concatenate([Av[0], Cv[0]], 0).astype(BF)
    out['t1e'] = np.concatenate([Av[1], Cv[1]], 0).astype(BF)
    out['t2'] = np.concatenate([Wn, Bn], 0).astype(BF)
    out['negones'] = np.full((33, H), -1.0, BF)

    # gathered row-mean tables, coefficient slots [p, b*52 + k*4 + t]
    for br, (na, ncn) in ((0, ('gmaw', 'gmcw')), (1, ('gmae', 'gmce'))):
        ta = np.zeros((128, 104), np.float32)
        tcn = np.zeros((128, 104), np.float32)
        rmA, rmC = Av[br].mean(1), Cv[br].mean(1)
        for t in range(D):
            va = np.zeros(PAD, np.float32)
            vc = np.zeros(PAD, np.float32)
            va[valid] = rmA[p_t[valid, t]]
            vc[valid] = rmC[p_t[valid, t]]
            for b in range(2):
                ta[cs_p, b * 52 + cs_k * 4 + t] = va
                tcn[cs_p, b * 52 + cs_k * 4 + t] = vc
        out[na], out[ncn] = ta, tcn
    g0 = np.zeros((128, 26), np.float32)
    gB = np.zeros((128, 26), np.float32)
    rmW, rmB = Wn.mean(1), Bn.mean(1)
    v0 = np.zeros(PAD, np.float32)
    vB = np.zeros(PAD, np.float32)
    v0[valid] = rmW[p_t[valid, 0]]
    vB[valid] = rmB[p_t[valid, 0]]
    for b in range(2):
        g0[cs_p, b * 13 + cs_k] = v0
        gB[cs_p, b * 13 + cs_k] = vB
    out['gm0'], out['gmb'] = g0, gB

    Wowg = inp['gamma_w'].astype(np.float32) * inp['Wow'][:, 0].astype(np.float32)
    out['wowgr'] = np.tile(Wowg[None, :], (128, NBLK)).astype(BF)

    # forest mask in transposed slots: m01[p', kk*100+f] = M01[kk*128+p', f]
    swr = inp['swr'].astype(np.int64)
    M01 = np.zeros((PAD, N_FOREST), np.float32)
    for f in range(N_FOREST):
        r = swr[f]
        s = (r % 25) * 64 + (r // 25)
        M01[s, f] = 1.0
    out['m01'] = M01.reshape(NBLK, 128, N_FOREST).transpose(1, 0, 2).reshape(128, NBLK * N_FOREST).astype(BF)

    tailc = np.zeros((128, 548), np.float32)
    tailc[0:1, 420:548] = 1.0
    tailc[:, 0:128] = inp['gamma_E'].astype(np.float32)[:, None] * inp['WoE'].astype(np.float32)
    tailc[:, 128:256] = inp['g1'].astype(np.float32)[:, None] * inp['W1'].astype(np.float32)
    W2p = inp['g2'].astype(np.float32)[:, None] * inp['W2'].astype(np.float32)
    tailc[:, 256:272] = np.concatenate([W2p, np.zeros((H, 6), np.float32)], 1)
    tailc[:, 272:273] = (inp['be1'] @ inp['W1'] + inp['b1'])[:, None].astype(np.float32)
    tailc[:, 273:274] = 1.0
    tailc[:, 274:275] = EPS
    tailc[:, 275:276] = float(inp['beta_w'] @ inp['Wow'][:, 0] + inp['bow'][0])
    b2p = (inp['be2'] @ inp['W2'] + inp['b2']).astype(np.float32) / N_FOREST
    tailc[0:16, 276:277] = np.concatenate([b2p, np.zeros(6, np.float32)])[:, None]
    tailc[0:1, 277:405] = (inp['beta_E'] @ inp['WoE'] + inp['boE'])[None, :].astype(np.float32)
    out['tailc'] = tailc

    # bf16 tail tables: phi3 matmuls run in bf16 with the mean-correction
    # rank-1 terms; forest-mean 1/N folded into the last layer
    W2B = tailc[:, 256:272] / N_FOREST
    tailb = np.zeros((128, 673), BF)
    tailb[:, TB_WOE:TB_WOE + 128] = tailc[:, 0:128]
    tailb[:, TB_W1B:TB_W1B + 128] = tailc[:, 128:256]
    tailb[:, TB_W2B:TB_W2B + 16] = W2B
    tailb[:, TB_ONEHC:TB_ONEHC + 1] = 1.0 / H
    tailb[0:1, TB_ONE1:TB_ONE1 + 128] = 1.0
    tailb[0:1, TB_BOE2:TB_BOE2 + 128] = tailc[0:1, 277:405]
    tailb[0:1, TB_W1SN:TB_W1SN + 128] = -tailc[:, 128:256].sum(0)[None, :]
    tailb[0:1, TB_W2SN:TB_W2SN + 16] = -W2B.sum(0)[None, :].astype(np.float32)
    out['tailb'] = tailb
    return out


def _host_x(inp, bs):
    x = inp['x'].astype(np.float32)
    q_of = np.arange(NSEQ)
    cs_p, cs_k = q_of // NBLK, q_of % NBLK
    g = q_of // 64
    xd = np.zeros((128, 130), np.float32)
    for bi, b in enumerate(bs):
        for t in range(D):
            xd[cs_p, bi * 52 + cs_k * 4 + t] = x[b, 4 * g + t]
        xd[cs_p, 104 + bi * 13 + cs_k] = x[b, 4 * g]
    return {'xd': xd}


_H_SHAPES = {
    'gvh': ((128, 416), F32), 'xd': ((128, 130), F32),
    'oht0': ((128, W2), BF16), 'oht1': ((128, W2), BF16),
    'oht2': ((128, W2), BF16), 'oht3': ((128, W2), BF16),
    'oht0t': ((64, W2), BF16),
    't1w': ((128, 128), BF16), 't1e': ((128, 128), BF16),
    't2': ((128, 128), BF16), 'negones': ((33, 128), BF16),
    'gmaw': ((128, 104), F32), 'gmcw': ((128, 104), F32),
    'gmae': ((128, 104), F32), 'gmce': ((128, 104), F32),
    'gm0': ((128, 26), F32), 'gmb': ((128, 26), F32),
    'wowgr': ((128, NBLK * 128), BF16), 'm01': ((128, NBLK * 100), BF16),
    'tailc': ((128, 548), F32), 'tailb': ((128, 673), BF16),
}


def _vw(ap, off, dims):
    return bass.AP(tensor=ap.tensor, offset=ap.offset + off,
                   ap=[list(ap.ap[0])] + [[s, c] for (s, c) in dims])


def _vw1(ap):
    return bass.AP(tensor=ap.tensor, offset=ap.offset,
                   ap=[[ap.ap[0][0], 1]] + [list(d) for d in ap.ap[1:]])


def _dram_ap(handle, off, dims):
    return bass.AP(tensor=handle.tensor, offset=handle.offset + off,
                   ap=[[s, c] for (s, c) in dims])


def _build_nc():
    nc = bacc_mod.Bacc()
    dram = {k: nc.declare_dram_parameter(k, list(sh), dt, isOutput=False)
            for k, (sh, dt) in _H_SHAPES.items()}
    out_d = nc.declare_dram_parameter('out', [2, 16], F32, isOutput=True)
    rowt = [nc.dram_tensor(f'rowt{r}', [1, W2], BF16) for r in range(N_ROWS)]
    bounce = nc.dram_tensor('bounce', [1, 1400], F32)
    # export: coefficient tile [128, (b,k)] -> DRAM row at b*1664 + p*13 + k
    XDIMS = [(NBLK, 2), (1, NBLK)]           # src free dims (b, k)
    XDST = [(NBLK, 128), (PAD, 2), (1, NBLK)]  # dst dims (p, b, k)

    with TileContext(nc) as tc:
        with (
            tc.tile_pool(name='const', bufs=1) as cp,
            tc.tile_pool(name='work', bufs=1) as wk,
            tc.tile_pool(name='cpT', bufs=5) as cpT,
            tc.tile_pool(name='scr2', bufs=3) as scr2,
            tc.tile_pool(name='px', bufs=2, space='PSUM') as px,
            tc.tile_pool(name='pt', bufs=2, space='PSUM') as pt_pool,
            tc.tile_pool(name='pz', bufs=2, space='PSUM') as pz,
        ):
            tl = {}
            _early = ['gvh', 'xd', 'gmaw', 'gmcw', 'gmae', 'gmce', 'gm0',
                      'gmb', 'tailc', 'tailb', 't1w', 't1e', 't2', 'negones',
                      'm01', 'wowgr']
            _late = ['oht0', 'oht1', 'oht2', 'oht3', 'oht0t']
            for k in _early:
                sh, dt = _H_SHAPES[k]
                t = cp.tile(list(sh), dt, name=f'c_{k}')
                nc.sync.dma_start(out=t[...], in_=dram[k][...])
                tl[k] = t
            for k in _late:
                sh, dt = _H_SHAPES[k]
                t = cp.tile(list(sh), dt, name=f'c_{k}')
                nc.scalar.dma_start(out=t[...], in_=dram[k][...])
                tl[k] = t
            wrhs2 = wk.tile([128, W2], BF16, tag='wrhs2')
            nc.scalar.dma_start(out=wrhs2[64:128, :], in_=dram['oht0t'][...])
            touch = cp.tile([128, 1], F32, tag='touch')

            def _touch(src, eng=None):
                (eng or nc.vector).tensor_copy(touch[0:src.ap[0][1], 0:1], src)

            tailc = tl['tailc']
            WoEg = tailc[:, 0:128]
            W1p = tailc[:, 128:256]
            W2pc = tailc[:, 256:272]
            b1p = tailc[:, 272:273]
            ones = tailc[:, 273:274]
            eps_sb = tailc[:, 274:275]
            bow2 = tailc[:, 275:276]
            b2p = tailc[0:16, 276:277]
            boE2 = tailc[0:1, 277:405]
            onesrow = tailc[0:1, 420:548]
            oht = [tl[f'oht{t}'] for t in range(D)]

            _touch(tailc[:, 0:1])
            _touch(tl['xd'][:, 0:1])
            _touch(tl['gvh'][:, 0:1])

            # ---------------- P1: softmax per batch -> m/a ----------------
            m_all = wk.tile([128, 208], F32, tag='m_all')
            a_all = wk.tile([128, 208], F32, tag='a_all')
            # bf16, export layout col = br*104 + t*26 + b*13 + k
            m_bf = wk.tile([128, 208], BF16, tag='m_bf')
            a_bf = wk.tile([128, 208], BF16, tag='a_bf')
            x0_bf = wk.tile([128, 26], BF16, tag='x0_bf')
            xt_rows = tl['xd'][:, 0:104]
            x0_rows = tl['xd'][:, 104:130]
            nc.scalar.copy(x0_bf[:, :], x0_rows)
            gvv = lambda kind: _vw(tl['gvh'][:, :], kind * 52, [(208, 2), (4, NBLK), (1, 4)])
            for b in range(2):
                xtv = _vw(xt_rows, b * 52, [(0, 2), (4, NBLK), (1, 4)])
                x0v = _vw(x0_rows, 104 - 104 + b * NBLK, [(0, 2), (1, NBLK), (0, 4)])
                t1 = wk.tile([128, 104], F32, name=f't1_{b}')
                t2 = wk.tile([128, 104], F32, name=f't2_{b}')
                nc.vector.tensor_mul(t1[:, :], gvv(0), xtv)
                nc.gpsimd.tensor_mul(t2[:, :], gvv(2), xtv)
                nc.vector.tensor_add(t1[:, :], t1[:, :], gvv(1))
                nc.gpsimd.tensor_add(t2[:, :], t2[:, :], gvv(3))
                nc.vector.tensor_mul(t1[:, :], t1[:, :], x0v)
                nc.vector.tensor_add(t1[:, :], t1[:, :], t2[:, :])
                e = wk.tile([128, 104], F32, name=f'e_{b}')
                nc.scalar.activation(e[:, :], t1[:, :], AF.Exp, bias=0.0, scale=S128)
                esum = wk.tile([128, 26], F32, name=f'esum_{b}')
                nc.vector.tensor_reduce(esum[:, :], _vw(e[:, :], 0, [(4, 26), (1, 4)]),
                                        mybir.AxisListType.X, OP.add)
                nc.vector.reciprocal(esum[:, :], esum[:, :])
                asl = a_all[:, b * 104:(b + 1) * 104]
                msl = m_all[:, b * 104:(b + 1) * 104]
                nc.vector.tensor_mul(asl, e[:, :], _vw(esum[:, :], 0, [(1, 26), (0, 4)]))
                nc.vector.tensor_mul(msl, asl, xtv)
                for src, dst in ((a_all, a_bf), (m_all, m_bf)):
                    nc.scalar.copy(
                        _vw(dst[:, :], b * 13, [(104, 2), (26, 4), (1, 13)]),
                        _vw(src[:, :], b * 104, [(52, 2), (1, 4), (4, 13)]))

            # ---------------- P3/P4: export rows + broadcast imports -------
            # each (br, t) pair stays on one HW queue so its DRAM write
            # is ordered before the broadcast read of the same row
            qeng = (nc.sync, nc.scalar)
            qof = lambda br, t: qeng[(br * 4 + t) % 2]
            # export + import of each (br, t) issued back-to-back on one
            # queue so the first coefT lands right after its own rows
            coefT = {}
            for br in range(2):
                for t in range(D):
                    for src, ridx in ((a_bf, ROW_A(br, t)), (m_bf, ROW_M(br, t))):
                        qof(br, t).dma_start(
                            out=_dram_ap(rowt[ridx][0, 0:1], 0, XDST),
                            in_=_vw(src[:, :], br * 104 + t * 26, XDIMS))
                    ct = cpT.tile([128, W2], BF16, name='ct')
                    # spread the bandwidth-heavy broadcast imports over a
                    # third queue (gpsimd); dense 64-descriptor pattern is
                    # cheap for SWDGE, and cross-queue W->R is semaphored
                    iq = (nc.sync, nc.scalar, nc.gpsimd)[(br * 4 + t) % 3]
                    iq.dma_start(out=ct[0:64, :],
                                 in_=_dram_ap(rowt[ROW_M(br, t)][0, 0:1], 0, [(0, 64), (1, W2)]))
                    iq.dma_start(out=ct[64:128, :],
                                 in_=_dram_ap(rowt[ROW_A(br, t)][0, 0:1], 0, [(0, 64), (1, W2)]))
                    coefT[(br, t)] = ct
            # ---------------- P2: mu via gathered row-means ----------------
            mu = wk.tile([128, 52], F32, tag='mu')       # col = br*26 + b*13 + k
            mu_bf = wk.tile([128, 52], BF16, tag='mu_bf')
            for br, (ga, gc) in enumerate((('gmaw', 'gmcw'), ('gmae', 'gmce'))):
                pm = wk.tile([128, 104], F32, name=f'pm_{br}')
                pa = wk.tile([128, 104], F32, name=f'pa_{br}')
                mview = _vw(m_all[:, :], br * 52, [(104, 2), (1, 52)])
                aview = _vw(a_all[:, :], br * 52, [(104, 2), (1, 52)])
                nc.vector.tensor_mul(pm[:, :], mview, tl[ga][:, :])
                nc.gpsimd.tensor_mul(pa[:, :], aview, tl[gc][:, :])
                nc.vector.tensor_add(pm[:, :], pm[:, :], pa[:, :])
                musl = mu[:, br * 26:(br + 1) * 26]
                nc.vector.tensor_reduce(musl, _vw(pm[:, :], 0, [(52, 2), (4, 13), (1, 4)]),
                                        mybir.AxisListType.X, OP.add)
                x0g = wk.tile([128, 26], F32, name=f'x0g_{br}')
                nc.gpsimd.tensor_mul(x0g[:, :], x0_rows, tl['gm0'][:, :])
                nc.vector.tensor_add(musl, musl, x0g[:, :])
                nc.vector.tensor_add(musl, musl, tl['gmb'][:, :])
                nc.scalar.copy(mu_bf[:, br * 26:(br + 1) * 26], musl)

            nc.gpsimd.dma_start(out=_dram_ap(rowt[ROW_X0][0, 0:1], 0, XDST),
                                in_=_vw(x0_bf[:, :], 0, XDIMS))
            for br in range(2):
                nc.gpsimd.dma_start(
                    out=_dram_ap(rowt[ROW_MU(br)][0, 0:1], 0, XDST),
                    in_=_vw(mu_bf[:, :], br * 26, XDIMS))

            murows = wk.tile([33, W2], BF16, tag='murows')
            x0bc_t = wk.tile([64, W2], BF16, tag='x0bc_t')
            x0bc = x0bc_t[:, :]
            murow = {0: murows[0:1, :], 1: murows[32:33, :]}
            nc.gpsimd.dma_start(out=x0bc,
                                in_=_dram_ap(rowt[ROW_X0][0, 0:1], 0, [(0, 64), (1, W2)]))
            for br in range(2):
                nc.gpsimd.dma_start(out=murow[br],
                                    in_=_dram_ap(rowt[ROW_MU(br)][0, 0:1], 0, [(0, 1), (1, W2)]))

            # ---------------- P5: SS1 scatter build ------------------------
            # coefT tiles come from a 2-deep ring per branch: import of layer
            # t+2 waits (WAR) on the mul of layer t.
            veng = (nc.vector, nc.gpsimd)
            SS1 = {}
            sstmp = {}
            for br in range(2):
                ss = wk.tile([128, W2], BF16, name=f'ss1_{br}')
                tmpa = wk.tile([128, W2], BF16, name=f'sstmpa_{br}')
                tmpb = wk.tile([128, W2], BF16, name=f'sstmpb_{br}')
                SS1[br] = ss
                sstmp[br] = (tmpa, tmpb)
            for br in range(2):
                eng = nc.vector
                ss = SS1[br]
                tmpa, tmpb = sstmp[br]
                dsts = (ss, tmpa, tmpb, tmpa)
                for t in range(D):
                    ct = coefT[(br, t)]
                    _touch(ct[:, 0:1], eng)
                    eng.tensor_mul(dsts[t][:, :], oht[t][:, :], ct[:, :])
                    if t == 1:
                        eng.tensor_add(ss[:, :], ss[:, :], tmpa[:, :])
                eng.tensor_add(ss[:, :], ss[:, :], tmpb[:, :])
                eng.tensor_add(ss[:, :], ss[:, :], tmpa[:, :])
            _touch(x0bc[0:1, 0:1])
            nc.gpsimd.tensor_mul(wrhs2[0:64, :], tl['oht0t'][:, :], x0bc)
            for br in range(2):
                _touch(murow[br][0:1, 0:1])

            # ---------------- P6/P7: x_inT matmuls + copy out --------------
            t1t = {0: tl['t1w'], 1: tl['t1e']}
            xs = {}
            for br in range(2):
                x_s = wk.tile([128, W2], BF16, name=f'xs_{br}')
                xs[br] = x_s
                for ci, (c0, c1) in enumerate(CHUNKS):
                    pch = px.tile([128, CHUNK], F32, name='pxc')[:, 0:c1 - c0]
                    nc.tensor.matmul(pch[:, :], t1t[br][:, :], SS1[br][:, c0:c1],
                                     start=True, stop=False)
                    nc.tensor.matmul(pch[:, :], tl['t2'][:, :], wrhs2[:, c0:c1],
                                     start=False, stop=False)
                    nc.tensor.matmul(pch[:, :], tl['negones'][32 * br:32 * br + 1, :],
                                     murow[br][0:1, c0:c1], start=False, stop=True)
                    nc.scalar.copy(x_s[:, c0:c1], pch[:, :])

            # ---------------- P8: DMA transpose to seq-slot layout ---------
            xin = {}
            for br in range(2):
                for b in range(2):
                    xt_ = wk.tile([128, NBLK, 128], BF16, name=f'xin_{br}_{b}')
                    qeng[br].dma_start(out=xt_[:, :, :],
                                       in_=xs[br][:, b * PAD:(b + 1) * PAD],
                                       transpose=True)
                    xin[(br, b)] = xt_

            # ---------------- P9: stats ------------------------------------
            ssq = wk.tile([128, 52], F32, tag='ssq')      # col = br*26 + b*13 + kk
            wraw = wk.tile([128, 26], F32, tag='wraw')
            for br in range(2):
                for b in range(2):
                    xt_ = xin[(br, b)]
                    _touch(xt_[:, 0, 0:1], veng[br])
                    sq = scr2.tile([128, PAD], BF16, name='sqt')
                    nc.scalar.square(sq[:, :], _vw(xt_[:, :, :], 0, [(1, PAD)]))
                    nc.vector.tensor_reduce(
                        ssq[:, br * 26 + b * 13: br * 26 + (b + 1) * 13],
                        _vw(sq[:, :], 0, [(128, NBLK), (1, 128)]),
                        mybir.AxisListType.X, OP.add)
            for b in range(2):
                wx = scr2.tile([128, PAD], BF16, name='sqt')
                nc.gpsimd.tensor_mul(wx[:, :], _vw(xin[(0, b)][:, :, :], 0, [(1, PAD)]),
                                     tl['wowgr'][:, :])
                nc.vector.tensor_reduce(wraw[:, b * 13:(b + 1) * 13],
                                        _vw(wx[:, :], 0, [(128, NBLK), (1, 128)]),
                                        mybir.AxisListType.X, OP.add)
            stdv = wk.tile([128, 52], F32, tag='stdv')
            nc.scalar.activation(stdv[:, :], ssq[:, :], AF.Sqrt,
                                 bias=eps_sb, scale=1.0 / H)
            rstd = wk.tile([128, 52], F32, tag='rstd')
            nc.vector.reciprocal(rstd[:, :], stdv[:, :])
            o2 = wk.tile([128, 26], F32, tag='o2')
            nc.vector.tensor_mul(o2[:, :], wraw[:, :], rstd[:, 0:26])
            expw = wk.tile([128, 26], F32, tag='expw')
            nc.scalar.activation(expw[:, :], o2[:, :], AF.Exp, bias=bow2, scale=1.0)
            er = wk.tile([128, 26], F32, tag='er')
            nc.vector.tensor_mul(er[:, :], expw[:, :], rstd[:, 26:52])
            std_bf = wk.tile([128, 26], BF16, tag='std_bf')
            nc.scalar.copy(std_bf[:, :], stdv[:, 26:52])
            er_bf = wk.tile([128, 26], BF16, tag='er_bf')
            nc.scalar.copy(er_bf[:, :], er[:, :])

            # ---------------- P10: forest ----------------------------------
            # main_c = sum_seq er*E_c*M01 via 13 accumulating matmuls; the
            # forest softmax denominator reuses M01er: z = sum std_E*M01er
            psC, psZ = {}, {}
            for b in range(2):
                m01er = wk.tile([128, NBLK * 100], BF16, name=f'm01er_{b}')
                nc.vector.tensor_mul(
                    m01er[:, :], tl['m01'][:, :],
                    _vw(er_bf[:, :], b * NBLK, [(1, NBLK), (0, 100)]))
                pc = pt_pool.tile([128, 200], F32, name='tailps')[:, 0:100]
                pzt = pz.tile([16, 200], F32, name='rowps')[0:1, 0:100]
                for k in range(NBLK):
                    nc.tensor.matmul(pc, xin[(1, b)][:, k, :],
                                     m01er[:, k * 100:(k + 1) * 100],
                                     start=(k == 0), stop=(k == NBLK - 1))
                    nc.tensor.matmul(pzt, std_bf[:, b * 13 + k: b * 13 + k + 1],
                                     m01er[:, k * 100:(k + 1) * 100],
                                     start=(k == 0), stop=(k == NBLK - 1),
                                     skip_group_check=True)
                psC[b], psZ[b] = pc, pzt

            # ---------------- P11: tail (batch-pair fused) -----------------
            # LN is invariant to any positive per-column scale, so the
            # forest-softmax denominator z never needs to divide main:
            # LN1((WoEg@main + boE2*z) / z) == LN1(WoEg@main + boE2*z).
            # Each LN+Linear is refactored as matmul-first with a rank-1
            # mean correction: W^T@LN(V) = (W^T@V - colsum(W) x mu) * rstd,
            # so the big matmul runs before the stats finish. All tail
            # matmuls in bf16; the 1/n_forest mean is folded into W2.
            tailb = tl['tailb']
            msb = wk.tile([128, 200], BF16, tag='msb')
            z_sb = wk.tile([1, 200], BF16, tag='z_sb')
            for b in range(2):
                nc.scalar.copy(msb[:, b * 100:(b + 1) * 100], psC[b])
                nc.vector.tensor_copy(z_sb[0:1, b * 100:(b + 1) * 100], psZ[b])
            pt = pt_pool.tile([128, 200], F32, name='tailps')
            nc.tensor.matmul(pt[:, :], tailb[:, TB_WOE:TB_WOE + 128], msb[:, :],
                             start=True, stop=False)
            nc.tensor.matmul(pt[:, :], tailb[0:1, TB_BOE2:TB_BOE2 + 128],
                             z_sb[0:1, :], start=False, stop=True,
                             skip_group_check=True)

            onehc = tailb[:, TB_ONEHC:TB_ONEHC + 1]
            ones1r = tailb[0:1, TB_ONE1:TB_ONE1 + 128]

            def ln_lin(Vp, Wb, wsnr, nout, nm):
                # Vp: [128,200] PSUM; returns [nout,200] PSUM of
                # (W^T@V - colsum(W) x mu) * rstd  awaiting bias+act
                Vb = wk.tile([128, 400], BF16, name=f'vb_{nm}')
                nc.scalar.copy(Vb[:, 0:200], Vp)
                nc.gpsimd.tensor_mul(Vb[:, 200:400], Vb[:, 0:200], Vb[:, 0:200])
                cs = pz.tile([16, 400], F32, name='csps')[0:1, :]
                nc.tensor.matmul(cs, onehc, Vb[:, :], start=True, stop=True)
                st = wk.tile([1, 600], F32, name=f'st_{nm}')
                nc.vector.tensor_copy(st[0:1, 400:600], cs[0:1, 0:200])
                nc.vector.tensor_mul(st[0:1, 200:400], st[0:1, 400:600],
                                     st[0:1, 400:600])
                nc.vector.tensor_sub(st[0:1, 200:400], cs[0:1, 200:400],
                                     st[0:1, 200:400])
                nc.scalar.activation(st[0:1, 200:400], st[0:1, 200:400],
                                     AF.Sqrt, bias=_vw1(eps_sb), scale=1.0)
                nc.vector.reciprocal(st[0:1, 200:400], st[0:1, 200:400])
                stb = wk.tile([1, 400], BF16, name=f'stb_{nm}')
                nc.scalar.copy(stb[0:1, 0:200], st[0:1, 400:600])
                nc.scalar.copy(stb[0:1, 200:400], st[0:1, 200:400])
                mb = px.tile([128, CHUNK], F32, name='pxc')[:, 0:200]
                nc.tensor.matmul(mb, ones1r, stb[0:1, 200:400], start=True,
                                 stop=True)
                rbs = wk.tile([128, 200], BF16, name=f'rbs_{nm}')
                nc.scalar.copy(rbs[:, :], mb)
                Pp = pt_pool.tile([128, 200], F32, name='tailps')[0:nout, :]
                nc.tensor.matmul(Pp, Wb, Vb[:, 0:200], start=True, stop=False)
                nc.tensor.matmul(Pp, wsnr, stb[0:1, 0:200], start=False,
                                 stop=True, skip_group_check=True)
                q = wk.tile([128, 200], F32, name=f'q_{nm}')[0:nout, :]
                nc.vector.tensor_mul(q, Pp, rbs[0:nout, :])
                return q

            q1 = ln_lin(pt[:, :], tailb[:, TB_W1B:TB_W1B + 128],
                        tailb[0:1, TB_W1SN:TB_W1SN + 128], 128, 'l1')
            h1p = pt_pool.tile([128, 200], F32, name='tailps')
            nc.scalar.activation(h1p[:, :], q1, AF.Relu, bias=b1p, scale=1.0)
            q2 = ln_lin(h1p[:, :], tailb[:, TB_W2B:TB_W2B + 16],
                        tailb[0:1, TB_W2SN:TB_W2SN + 16], 16, 'l2')
            ob = wk.tile([16, 200], F32, tag='ob')
            nc.scalar.activation(ob[:, :], q2, AF.Identity, bias=b2p, scale=1.0)
            ored = wk.tile([16, 2], F32, tag='ored')
            nc.vector.tensor_reduce(ored[:, :], _vw(ob[:, :], 0, [(100, 2), (1, 100)]),
                                    mybir.AxisListType.X, OP.add)
            for b in range(2):
                nc.sync.dma_start(out=out_d[b, :], in_=ored[:, b:b + 1])
    nc.finalize()
    return nc


_NC_CACHE = {}


def kernel(**inputs):
    inp = {k: np.asarray(v) for k, v in inputs.items()}
    H_ = _host_precompute(inp)
    if 'nc' not in _NC_CACHE:
        _NC_CACHE['nc'] = _build_nc()
    nc = _NC_CACHE['nc']
    in_maps = []
    for c in range(NCORES):
        m = {k: np.ascontiguousarray(H_[k]) for k in H_}
        m.update({k: np.ascontiguousarray(v)
                  for k, v in _host_x(inp, (2 * c, 2 * c + 1)).items()})
        in_maps.append(m)
    res = run_bass_kernel_spmd(nc, in_maps, list(range(NCORES)))
    out = np.zeros((B, N_CLASS), np.float32)
    for c in range(NCORES):
        out[2 * c:2 * c + 2] = res.results[c]['out'][:, :N_CLASS]
    return out



# revision 49
# speedup vs baseline: 1.1358x; 1.0270x over previous
"""DOFENTransformer Trainium2 kernel, v2.

Data-parallel: 16 batches / 8 cores = one batch-PAIR per core. The
per-token attention-output rows (x_in = attn_out + residual) are built on
the TensorEngine instead of per-block vector FMA chains:

  x_inT[h, q] = T1^T @ SS1 + T2^T @ rhs2 - ones^T @ mu_row
    SS1[c | c+64, q] = one-hot scatter of softmax coefficients (m, a),
    built with 7 wide tensor_tensor ops per branch from host one-hot masks
    and DMA-broadcast coefficient rows; rhs2 carries the x0/residual
    terms. mu (row means) comes from host-gathered table means, so x_in
    arrives mean-centered and LayerNorm needs only a sum of squares.

Column q of x_inT holds rODT seq q. Coefficient-side tiles use slot
(p, k) = (q // 13, q % 13); x_inT is DMA-XBAR-transposed to seq-slot
layout (p', kk) = (q % 128, q // 128) for the forest contraction (PE) and
stats (strided reduces). Host tables are gathered per-slot so the two
layouts never mix on device. bf16 for all wide ops; fp32 stats + tail.
"""
import sys

for p in ('/opt/trn_rl_repo', '/root/.axon_site/_ro/trn_rl_repo'):
    if p not in sys.path:
        sys.path.insert(0, p)

import numpy as np
import ml_dtypes
import concourse.bass as bass
import concourse.bacc as bacc_mod
from concourse import mybir
from concourse.tile import TileContext
from concourse.bass_utils import run_bass_kernel_spmd

B, N_COL, N_COND, D, H = 16, 100, 64, 4, 128
N_FOREST, N_CLASS = 100, 10
NSEQ, NBLK, PAD = 1600, 13, 1664
W2 = 2 * PAD  # batch-pair width 3328
EPS = 1e-5
S128 = float(np.sqrt(128.0))
F32 = mybir.dt.float32
BF16 = mybir.dt.bfloat16
AF = mybir.ActivationFunctionType
OP = mybir.AluOpType
NCORES = 8
BF = ml_dtypes.bfloat16

CHUNK = 512
CHUNKS = [(c, min(c + CHUNK, W2)) for c in range(0, W2, CHUNK)]

ROW_M = lambda br, t: br * 4 + t
ROW_A = lambda br, t: 8 + br * 4 + t
ROW_X0 = 16
ROW_MU = lambda br: 17 + br
N_ROWS = 19

TB_WOE, TB_W1B, TB_W2B = 0, 128, 256
TB_ONEHC, TB_ONE1, TB_BOE2, TB_W1SN, TB_W2SN = 272, 273, 401, 529, 657


def _host_precompute(inp):
    sl = lambda i: slice(i * H, (i + 1) * H)
    Wn = inp['W_num'].reshape(N_COND, H).astype(np.float32)
    Bn = inp['b_num'].reshape(N_COND, H).astype(np.float32)
    Wqkv, bqkv = inp['Wqkv'].astype(np.float32), inp['bqkv'].astype(np.float32)
    perm = inp['perm'].astype(np.int64)
    A = Wn @ Wqkv
    C = Bn @ Wqkv + bqkv

    seq = np.arange(NSEQ)
    g = seq // 64
    j = seq % 64
    p_t = np.zeros((PAD, D), np.int64)
    for t in range(D):
        p_t[:NSEQ, t] = perm[4 * g + t, j]
    valid = np.arange(PAD) < NSEQ
    # coefficient-side slots: q = p*13 + k
    q_of = np.arange(PAD)
    cs_p, cs_k = q_of // NBLK, q_of % NBLK
    out = {}

    # logits Gram tables: gvh[p, br*208 + kind*52 + k*4 + t] for q=p*13+k
    gv = np.zeros((128, 416), np.float32)
    for br in range(2):
        Aq, Ak = A[:, sl(3 * br)], A[:, sl(3 * br + 1)]
        Cq, Ck = C[:, sl(3 * br)], C[:, sl(3 * br + 1)]
        Gt = (Aq @ Ak.T, Aq @ Ck.T, Cq @ Ak.T, Cq @ Ck.T)
        for kind in range(4):
            for t in range(D):
                v = np.zeros(PAD, np.float32)
                v[valid] = Gt[kind][p_t[valid, 0], p_t[valid, t]]
                gv[cs_p, br * 208 + kind * 52 + cs_k * 4 + t] = v
    out['gvh'] = gv

    # one-hot scatter-transpose masks [128, 3328], col = b*1664 + q
    for t in range(D):
        oh = np.zeros((N_COND, PAD), np.float32)
        for s in range(NSEQ):
            oh[p_t[s, t], s] += 1.0
        ohd = np.concatenate([oh, oh], 0)
        out[f'oht{t}'] = np.tile(ohd, (1, 2)).astype(BF)
    oh0 = np.zeros((N_COND, PAD), np.float32)
    for s in range(NSEQ):
        oh0[p_t[s, 0], s] += 1.0
    out['oht0t'] = np.tile(oh0, (1, 2)).astype(BF)  # [64, 3328]

    Av, Cv = {}, {}
    for br in range(2):
        WV, bV = Wqkv[:, sl(3 * br + 2)], bqkv[sl(3 * br + 2)]
        Av[br] = Wn @ WV
        Cv[br] = Bn @ WV + bV
    out['t1w'] = np.concatenate([Av[0], Cv[0]], 0).astype(BF)
    out['t1e'] = np.concatenate([Av[1], Cv[1]], 0).astype(BF)
    out['t2'] = np.concatenate([Wn, Bn], 0).astype(BF)
    out['negones'] = np.full((33, H), -1.0, BF)

    # gathered row-mean tables, coefficient slots [p, b*52 + k*4 + t]
    for br, (na, ncn) in ((0, ('gmaw', 'gmcw')), (1, ('gmae', 'gmce'))):
        ta = np.zeros((128, 104), np.float32)
        tcn = np.zeros((128, 104), np.float32)
        rmA, rmC = Av[br].mean(1), Cv[br].mean(1)
        for t in range(D):
            va = np.zeros(PAD, np.float32)
            vc = np.zeros(PAD, np.float32)
            va[valid] = rmA[p_t[valid, t]]
            vc[valid] = rmC[p_t[valid, t]]
            for b in range(2):
                ta[cs_p, b * 52 + cs_k * 4 + t] = va
                tcn[cs_p, b * 52 + cs_k * 4 + t] = vc
        out[na], out[ncn] = ta, tcn
    g0 = np.zeros((128, 26), np.float32)
    gB = np.zeros((128, 26), np.float32)
    rmW, rmB = Wn.mean(1), Bn.mean(1)
    v0 = np.zeros(PAD, np.float32)
    vB = np.zeros(PAD, np.float32)
    v0[valid] = rmW[p_t[valid, 0]]
    vB[valid] = rmB[p_t[valid, 0]]
    for b in range(2):
        g0[cs_p, b * 13 + cs_k] = v0
        gB[cs_p, b * 13 + cs_k] = vB
    out['gm0'], out['gmb'] = g0, gB

    Wowg = inp['gamma_w'].astype(np.float32) * inp['Wow'][:, 0].astype(np.float32)
    out['wowgr'] = np.tile(Wowg[None, :], (128, NBLK)).astype(BF)

    # forest mask in transposed slots: m01[p', kk*100+f] = M01[kk*128+p', f]
    swr = inp['swr'].astype(np.int64)
    M01 = np.zeros((PAD, N_FOREST), np.float32)
    for f in range(N_FOREST):
        r = swr[f]
        s = (r % 25) * 64 + (r // 25)
        M01[s, f] = 1.0
    out['m01'] = M01.reshape(NBLK, 128, N_FOREST).transpose(1, 0, 2).reshape(128, NBLK * N_FOREST).astype(BF)

    tailc = np.zeros((128, 548), np.float32)
    tailc[0:1, 420:548] = 1.0
    tailc[:, 0:128] = inp['gamma_E'].astype(np.float32)[:, None] * inp['WoE'].astype(np.float32)
    tailc[:, 128:256] = inp['g1'].astype(np.float32)[:, None] * inp['W1'].astype(np.float32)
    W2p = inp['g2'].astype(np.float32)[:, None] * inp['W2'].astype(np.float32)
    tailc[:, 256:272] = np.concatenate([W2p, np.zeros((H, 6), np.float32)], 1)
    tailc[:, 272:273] = (inp['be1'] @ inp['W1'] + inp['b1'])[:, None].astype(np.float32)
    tailc[:, 273:274] = 1.0
    tailc[:, 274:275] = EPS
    tailc[:, 275:276] = float(inp['beta_w'] @ inp['Wow'][:, 0] + inp['bow'][0])
    b2p = (inp['be2'] @ inp['W2'] + inp['b2']).astype(np.float32) / N_FOREST
    tailc[0:16, 276:277] = np.concatenate([b2p, np.zeros(6, np.float32)])[:, None]
    tailc[0:1, 277:405] = (inp['beta_E'] @ inp['WoE'] + inp['boE'])[None, :].astype(np.float32)
    out['tailc'] = tailc

    # bf16 tail tables: phi3 matmuls run in bf16 with the mean-correction
    # rank-1 terms; forest-mean 1/N folded into the last layer
    W2B = tailc[:, 256:272] / N_FOREST
    tailb = np.zeros((128, 673), BF)
    tailb[:, TB_WOE:TB_WOE + 128] = tailc[:, 0:128]
    tailb[:, TB_W1B:TB_W1B + 128] = tailc[:, 128:256]
    tailb[:, TB_W2B:TB_W2B + 16] = W2B
    tailb[:, TB_ONEHC:TB_ONEHC + 1] = 1.0 / H
    tailb[0:1, TB_ONE1:TB_ONE1 + 128] = 1.0
    tailb[0:1, TB_BOE2:TB_BOE2 + 128] = tailc[0:1, 277:405]
    tailb[0:1, TB_W1SN:TB_W1SN + 128] = -tailc[:, 128:256].sum(0)[None, :]
    tailb[0:1, TB_W2SN:TB_W2SN + 16] = -W2B.sum(0)[None, :].astype(np.float32)
    out['tailb'] = tailb
    return out


def _host_x(inp, bs):
    x = inp['x'].astype(np.float32)
    q_of = np.arange(NSEQ)
    cs_p, cs_k = q_of // NBLK, q_of % NBLK
    g = q_of // 64
    xd = np.zeros((128, 130), np.float32)
    for bi, b in enumerate(bs):
        for t in range(D):
            xd[cs_p, bi * 52 + cs_k * 4 + t] = x[b, 4 * g + t]
        xd[cs_p, 104 + bi * 13 + cs_k] = x[b, 4 * g]
    return {'xd': xd}


_H_SHAPES = {
    'gvh': ((128, 416), F32), 'xd': ((128, 130), F32),
    'oht0': ((128, W2), BF16), 'oht1': ((128, W2), BF16),
    'oht2': ((128, W2), BF16), 'oht3': ((128, W2), BF16),
    'oht0t': ((64, W2), BF16),
    't1w': ((128, 128), BF16), 't1e': ((128, 128), BF16),
    't2': ((128, 128), BF16), 'negones': ((33, 128), BF16),
    'gmaw': ((128, 104), F32), 'gmcw': ((128, 104), F32),
    'gmae': ((128, 104), F32), 'gmce': ((128, 104), F32),
    'gm0': ((128, 26), F32), 'gmb': ((128, 26), F32),
    'wowgr': ((128, NBLK * 128), BF16), 'm01': ((128, NBLK * 100), BF16),
    'tailc': ((128, 548), F32), 'tailb': ((128, 673), BF16),
}


def _vw(ap, off, dims):
    return bass.AP(tensor=ap.tensor, offset=ap.offset + off,
                   ap=[list(ap.ap[0])] + [[s, c] for (s, c) in dims])


def _vw1(ap):
    return bass.AP(tensor=ap.tensor, offset=ap.offset,
                   ap=[[ap.ap[0][0], 1]] + [list(d) for d in ap.ap[1:]])


def _dram_ap(handle, off, dims):
    return bass.AP(tensor=handle.tensor, offset=handle.offset + off,
                   ap=[[s, c] for (s, c) in dims])


def _build_nc():
    nc = bacc_mod.Bacc()
    dram = {k: nc.declare_dram_parameter(k, list(sh), dt, isOutput=False)
            for k, (sh, dt) in _H_SHAPES.items()}
    out_d = nc.declare_dram_parameter('out', [2, 16], F32, isOutput=True)
    rowt = [nc.dram_tensor(f'rowt{r}', [1, W2], BF16) for r in range(N_ROWS)]
    bounce = nc.dram_tensor('bounce', [1, 1400], F32)
    # export: coefficient tile [128, (b,k)] -> DRAM row at b*1664 + p*13 + k
    XDIMS = [(NBLK, 2), (1, NBLK)]           # src free dims (b, k)
    XDST = [(NBLK, 128), (PAD, 2), (1, NBLK)]  # dst dims (p, b, k)

    with TileContext(nc) as tc:
        with (
            tc.tile_pool(name='const', bufs=1) as cp,
            tc.tile_pool(name='work', bufs=1) as wk,
            tc.tile_pool(name='cpT', bufs=5) as cpT,
            tc.tile_pool(name='scr2', bufs=3) as scr2,
            tc.tile_pool(name='px', bufs=2, space='PSUM') as px,
            tc.tile_pool(name='pt', bufs=2, space='PSUM') as pt_pool,
            tc.tile_pool(name='pz', bufs=2, space='PSUM') as pz,
        ):
            tl = {}
            _early = ['gvh', 'xd', 'gmaw', 'gmcw', 'gmae', 'gmce', 'gm0',
                      'gmb', 'tailc', 'tailb', 't1w', 't1e', 't2', 'negones',
                      'm01', 'wowgr']
            _late = ['oht0', 'oht1', 'oht2', 'oht3']
            for k in _early:
                sh, dt = _H_SHAPES[k]
                t = cp.tile(list(sh), dt, name=f'c_{k}')
                nc.sync.dma_start(out=t[...], in_=dram[k][...])
                tl[k] = t
            for k in _late:
                sh, dt = _H_SHAPES[k]
                t = cp.tile(list(sh), dt, name=f'c_{k}')
                nc.scalar.dma_start(out=t[...], in_=dram[k][...])
                tl[k] = t
            wrhs2 = wk.tile([128, W2], BF16, tag='wrhs2')
            # oht0t == oht0[0:64] (the mask rows are duplicated), so both
            # wrhs2 halves come from the oht0 tile instead of extra DMAs
            pass
            touch = cp.tile([128, 1], F32, tag='touch')

            def _touch(src, eng=None):
                (eng or nc.vector).tensor_copy(touch[0:src.ap[0][1], 0:1], src)

            tailc = tl['tailc']
            WoEg = tailc[:, 0:128]
            W1p = tailc[:, 128:256]
            W2pc = tailc[:, 256:272]
            b1p = tailc[:, 272:273]
            ones = tailc[:, 273:274]
            eps_sb = tailc[:, 274:275]
            bow2 = tailc[:, 275:276]
            b2p = tailc[0:16, 276:277]
            boE2 = tailc[0:1, 277:405]
            onesrow = tailc[0:1, 420:548]
            oht = [tl[f'oht{t}'] for t in range(D)]

            _touch(tailc[:, 0:1])
            _touch(tl['xd'][:, 0:1])
            _touch(tl['gvh'][:, 0:1])

            # ---------------- P1: softmax per batch -> m/a ----------------
            m_all = wk.tile([128, 208], F32, tag='m_all')
            a_all = wk.tile([128, 208], F32, tag='a_all')
            # bf16, export layout col = br*104 + t*26 + b*13 + k
            m_bf = wk.tile([128, 208], BF16, tag='m_bf')
            a_bf = wk.tile([128, 208], BF16, tag='a_bf')
            x0_bf = wk.tile([128, 26], BF16, tag='x0_bf')
            xt_rows = tl['xd'][:, 0:104]
            x0_rows = tl['xd'][:, 104:130]
            nc.scalar.copy(x0_bf[:, :], x0_rows)
            gvv = lambda kind: _vw(tl['gvh'][:, :], kind * 52, [(208, 2), (4, NBLK), (1, 4)])
            for b in range(2):
                xtv = _vw(xt_rows, b * 52, [(0, 2), (4, NBLK), (1, 4)])
                x0v = _vw(x0_rows, 104 - 104 + b * NBLK, [(0, 2), (1, NBLK), (0, 4)])
                t1 = wk.tile([128, 104], F32, name=f't1_{b}')
                t2 = wk.tile([128, 104], F32, name=f't2_{b}')
                nc.vector.tensor_mul(t1[:, :], gvv(0), xtv)
                nc.gpsimd.tensor_mul(t2[:, :], gvv(2), xtv)
                nc.vector.tensor_add(t1[:, :], t1[:, :], gvv(1))
                nc.gpsimd.tensor_add(t2[:, :], t2[:, :], gvv(3))
                nc.vector.tensor_mul(t1[:, :], t1[:, :], x0v)
                nc.vector.tensor_add(t1[:, :], t1[:, :], t2[:, :])
                e = wk.tile([128, 104], F32, name=f'e_{b}')
                nc.scalar.activation(e[:, :], t1[:, :], AF.Exp, bias=0.0, scale=S128)
                esum = wk.tile([128, 26], F32, name=f'esum_{b}')
                nc.vector.tensor_reduce(esum[:, :], _vw(e[:, :], 0, [(4, 26), (1, 4)]),
                                        mybir.AxisListType.X, OP.add)
                nc.vector.reciprocal(esum[:, :], esum[:, :])
                asl = a_all[:, b * 104:(b + 1) * 104]
                msl = m_all[:, b * 104:(b + 1) * 104]
                nc.vector.tensor_mul(asl, e[:, :], _vw(esum[:, :], 0, [(1, 26), (0, 4)]))
                nc.vector.tensor_mul(msl, asl, xtv)
                for src, dst in ((a_all, a_bf), (m_all, m_bf)):
                    nc.scalar.copy(
                        _vw(dst[:, :], b * 13, [(104, 2), (26, 4), (1, 13)]),
                        _vw(src[:, :], b * 104, [(52, 2), (1, 4), (4, 13)]))

            # ---------------- P3/P4: export rows + broadcast imports -------
            # each (br, t) pair stays on one HW queue so its DRAM write
            # is ordered before the broadcast read of the same row
            qeng = (nc.sync, nc.scalar)
            qof = lambda br, t: qeng[(br * 4 + t) % 2]
            # export + import of each (br, t) issued back-to-back on one
            # queue so the first coefT lands right after its own rows
            coefT = {}
            for br in range(2):
                for t in range(D):
                    for src, ridx in ((a_bf, ROW_A(br, t)), (m_bf, ROW_M(br, t))):
                        qof(br, t).dma_start(
                            out=_dram_ap(rowt[ridx][0, 0:1], 0, XDST),
                            in_=_vw(src[:, :], br * 104 + t * 26, XDIMS))
                    ct = cpT.tile([128, W2], BF16, name='ct')
                    # spread the bandwidth-heavy broadcast imports over a
                    # third queue (gpsimd); dense 64-descriptor pattern is
                    # cheap for SWDGE, and cross-queue W->R is semaphored
                    iq = (nc.sync, nc.scalar, nc.gpsimd)[(br * 4 + t) % 3]
                    iq.dma_start(out=ct[0:64, :],
                                 in_=_dram_ap(rowt[ROW_M(br, t)][0, 0:1], 0, [(0, 64), (1, W2)]))
                    iq.dma_start(out=ct[64:128, :],
                                 in_=_dram_ap(rowt[ROW_A(br, t)][0, 0:1], 0, [(0, 64), (1, W2)]))
                    coefT[(br, t)] = ct
            # ---------------- P2: mu via gathered row-means ----------------
            mu = wk.tile([128, 52], F32, tag='mu')       # col = br*26 + b*13 + k
            mu_bf = wk.tile([128, 52], BF16, tag='mu_bf')
            for br, (ga, gc) in enumerate((('gmaw', 'gmcw'), ('gmae', 'gmce'))):
                pm = wk.tile([128, 104], F32, name=f'pm_{br}')
                pa = wk.tile([128, 104], F32, name=f'pa_{br}')
                mview = _vw(m_all[:, :], br * 52, [(104, 2), (1, 52)])
                aview = _vw(a_all[:, :], br * 52, [(104, 2), (1, 52)])
                nc.vector.tensor_mul(pm[:, :], mview, tl[ga][:, :])
                nc.gpsimd.tensor_mul(pa[:, :], aview, tl[gc][:, :])
                nc.vector.tensor_add(pm[:, :], pm[:, :], pa[:, :])
                musl = mu[:, br * 26:(br + 1) * 26]
                nc.vector.tensor_reduce(musl, _vw(pm[:, :], 0, [(52, 2), (4, 13), (1, 4)]),
                                        mybir.AxisListType.X, OP.add)
                x0g = wk.tile([128, 26], F32, name=f'x0g_{br}')
                nc.gpsimd.tensor_mul(x0g[:, :], x0_rows, tl['gm0'][:, :])
                nc.vector.tensor_add(musl, musl, x0g[:, :])
                nc.vector.tensor_add(musl, musl, tl['gmb'][:, :])
                nc.scalar.copy(mu_bf[:, br * 26:(br + 1) * 26], musl)

            nc.gpsimd.dma_start(out=_dram_ap(rowt[ROW_X0][0, 0:1], 0, XDST),
                                in_=_vw(x0_bf[:, :], 0, XDIMS))
            for br in range(2):
                nc.gpsimd.dma_start(
                    out=_dram_ap(rowt[ROW_MU(br)][0, 0:1], 0, XDST),
                    in_=_vw(mu_bf[:, :], br * 26, XDIMS))

            murows = wk.tile([33, W2], BF16, tag='murows')
            x0bc_t = wk.tile([64, W2], BF16, tag='x0bc_t')
            x0bc = x0bc_t[:, :]
            murow = {0: murows[0:1, :], 1: murows[32:33, :]}
            nc.gpsimd.dma_start(out=x0bc,
                                in_=_dram_ap(rowt[ROW_X0][0, 0:1], 0, [(0, 64), (1, W2)]))
            for br in range(2):
                nc.gpsimd.dma_start(out=murow[br],
                                    in_=_dram_ap(rowt[ROW_MU(br)][0, 0:1], 0, [(0, 1), (1, W2)]))

            # ---------------- P5: SS1 scatter build ------------------------
            # coefT tiles come from a 2-deep ring per branch: import of layer
            # t+2 waits (WAR) on the mul of layer t.
            veng = (nc.vector, nc.gpsimd)
            SS1 = {}
            sstmp = {}
            for br in range(2):
                ss = wk.tile([128, W2], BF16, name=f'ss1_{br}')
                tmpa = wk.tile([128, W2], BF16, name=f'sstmpa_{br}')
                tmpb = wk.tile([128, W2], BF16, name=f'sstmpb_{br}')
                SS1[br] = ss
                sstmp[br] = (tmpa, tmpb)
            for br in range(2):
                eng = nc.vector
                ss = SS1[br]
                tmpa, tmpb = sstmp[br]
                dsts = (ss, tmpa, tmpb, tmpa)
                for t in range(D):
                    ct = coefT[(br, t)]
                    _touch(ct[:, 0:1], eng)
                    eng.tensor_mul(dsts[t][:, :], oht[t][:, :], ct[:, :])
                    if t == 1:
                        eng.tensor_add(ss[:, :], ss[:, :], tmpa[:, :])
                eng.tensor_add(ss[:, :], ss[:, :], tmpb[:, :])
                eng.tensor_add(ss[:, :], ss[:, :], tmpa[:, :])
            _touch(x0bc[0:1, 0:1])
            nc.scalar.copy(wrhs2[64:128, :], tl['oht0'][64:128, :])
            nc.gpsimd.tensor_mul(wrhs2[0:64, :], tl['oht0'][0:64, :], x0bc)
            for br in range(2):
                _touch(murow[br][0:1, 0:1])

            # ---------------- P6/P7: x_inT matmuls + copy out --------------
            t1t = {0: tl['t1w'], 1: tl['t1e']}
            xs = {}
            for br in range(2):
                x_s = wk.tile([128, W2], BF16, name=f'xs_{br}')
                xs[br] = x_s
                for ci, (c0, c1) in enumerate(CHUNKS):
                    pch = px.tile([128, CHUNK], F32, name='pxc')[:, 0:c1 - c0]
                    nc.tensor.matmul(pch[:, :], t1t[br][:, :], SS1[br][:, c0:c1],
                                     start=True, stop=False)
                    nc.tensor.matmul(pch[:, :], tl['t2'][:, :], wrhs2[:, c0:c1],
                                     start=False, stop=False)
                    nc.tensor.matmul(pch[:, :], tl['negones'][32 * br:32 * br + 1, :],
                                     murow[br][0:1, c0:c1], start=False, stop=True)
                    nc.scalar.copy(x_s[:, c0:c1], pch[:, :])

            # ---------------- P8: DMA transpose to seq-slot layout ---------
            xin = {}
            for br in range(2):
                for b in range(2):
                    xt_ = wk.tile([128, NBLK, 128], BF16, name=f'xin_{br}_{b}')
                    qeng[br].dma_start(out=xt_[:, :, :],
                                       in_=xs[br][:, b * PAD:(b + 1) * PAD],
                                       transpose=True)
                    xin[(br, b)] = xt_

            # ---------------- P9: stats ------------------------------------
            ssq = wk.tile([128, 52], F32, tag='ssq')      # col = br*26 + b*13 + kk
            wraw = wk.tile([128, 26], F32, tag='wraw')
            for br in range(2):
                for b in range(2):
                    xt_ = xin[(br, b)]
                    _touch(xt_[:, 0, 0:1], veng[br])
                    sq = scr2.tile([128, PAD], BF16, name='sqt')
                    nc.scalar.square(sq[:, :], _vw(xt_[:, :, :], 0, [(1, PAD)]))
                    nc.vector.tensor_reduce(
                        ssq[:, br * 26 + b * 13: br * 26 + (b + 1) * 13],
                        _vw(sq[:, :], 0, [(128, NBLK), (1, 128)]),
                        mybir.AxisListType.X, OP.add)
            for b in range(2):
                wx = scr2.tile([128, PAD], BF16, name='sqt')
                nc.gpsimd.tensor_mul(wx[:, :], _vw(xin[(0, b)][:, :, :], 0, [(1, PAD)]),
                                     tl['wowgr'][:, :])
                nc.vector.tensor_reduce(wraw[:, b * 13:(b + 1) * 13],
                                        _vw(wx[:, :], 0, [(128, NBLK), (1, 128)]),
                                        mybir.AxisListType.X, OP.add)
            stdv = wk.tile([128, 52], F32, tag='stdv')
            nc.scalar.activation(stdv[:, :], ssq[:, :], AF.Sqrt,
                                 bias=eps_sb, scale=1.0 / H)
            rstd = wk.tile([128, 52], F32, tag='rstd')
            nc.vector.reciprocal(rstd[:, :], stdv[:, :])
            o2 = wk.tile([128, 26], F32, tag='o2')
            nc.vector.tensor_mul(o2[:, :], wraw[:, :], rstd[:, 0:26])
            expw = wk.tile([128, 26], F32, tag='expw')
            nc.scalar.activation(expw[:, :], o2[:, :], AF.Exp, bias=bow2, scale=1.0)
            er = wk.tile([128, 26], F32, tag='er')
            nc.vector.tensor_mul(er[:, :], expw[:, :], rstd[:, 26:52])
            std_bf = wk.tile([128, 26], BF16, tag='std_bf')
            nc.scalar.copy(std_bf[:, :], stdv[:, 26:52])
            er_bf = wk.tile([128, 26], BF16, tag='er_bf')
            nc.scalar.copy(er_bf[:, :], er[:, :])

            # ---------------- P10: forest ----------------------------------
            # main_c = sum_seq er*E_c*M01 via 13 accumulating matmuls; the
            # forest softmax denominator reuses M01er: z = sum std_E*M01er
            psC, psZ = {}, {}
            for b in range(2):
                m01er = wk.tile([128, NBLK * 100], BF16, name=f'm01er_{b}')
                nc.vector.tensor_mul(
                    m01er[:, :], tl['m01'][:, :],
                    _vw(er_bf[:, :], b * NBLK, [(1, NBLK), (0, 100)]))
                pc = pt_pool.tile([128, 200], F32, name='tailps')[:, 0:100]
                pzt = pz.tile([16, 200], F32, name='rowps')[0:1, 0:100]
                for k in range(NBLK):
                    nc.tensor.matmul(pc, xin[(1, b)][:, k, :],
                                     m01er[:, k * 100:(k + 1) * 100],
                                     start=(k == 0), stop=(k == NBLK - 1))
                    nc.tensor.matmul(pzt, std_bf[:, b * 13 + k: b * 13 + k + 1],
                                     m01er[:, k * 100:(k + 1) * 100],
                                     start=(k == 0), stop=(k == NBLK - 1),
                                     skip_group_check=True)
                psC[b], psZ[b] = pc, pzt

            # ---------------- P11: tail (batch-pair fused) -----------------
            # LN is invariant to any positive per-column scale, so the
            # forest-softmax denominator z never needs to divide main:
            # LN1((WoEg@main + boE2*z) / z) == LN1(WoEg@main + boE2*z).
            # Each LN+Linear is refactored as matmul-first with a rank-1
            # mean correction: W^T@LN(V) = (W^T@V - colsum(W) x mu) * rstd,
            # so the big matmul runs before the stats finish. All tail
            # matmuls in bf16; the 1/n_forest mean is folded into W2.
            tailb = tl['tailb']
            msb = wk.tile([128, 200], BF16, tag='msb')
            z_sb = wk.tile([1, 200], BF16, tag='z_sb')
            for b in range(2):
                nc.scalar.copy(msb[:, b * 100:(b + 1) * 100], psC[b])
                nc.vector.tensor_copy(z_sb[0:1, b * 100:(b + 1) * 100], psZ[b])
            pt = pt_pool.tile([128, 200], F32, name='tailps')
            nc.tensor.matmul(pt[:, :], tailb[:, TB_WOE:TB_WOE + 128], msb[:, :],
                             start=True, stop=False)
            nc.tensor.matmul(pt[:, :], tailb[0:1, TB_BOE2:TB_BOE2 + 128],
                             z_sb[0:1, :], start=False, stop=True,
                             skip_group_check=True)

            onehc = tailb[:, TB_ONEHC:TB_ONEHC + 1]
            ones1r = tailb[0:1, TB_ONE1:TB_ONE1 + 128]

            def ln_lin(Vp, Wb, wsnr, nout, nm):
                # Vp: [128,200] PSUM; returns [nout,200] PSUM of
                # (W^T@V - colsum(W) x mu) * rstd  awaiting bias+act
                Vb = wk.tile([128, 400], BF16, name=f'vb_{nm}')
                nc.scalar.copy(Vb[:, 0:200], Vp)
                nc.gpsimd.tensor_mul(Vb[:, 200:400], Vb[:, 0:200], Vb[:, 0:200])
                cs = pz.tile([16, 400], F32, name='csps')[0:1, :]
                nc.tensor.matmul(cs, onehc, Vb[:, :], start=True, stop=True)
                st = wk.tile([1, 600], F32, name=f'st_{nm}')
                nc.vector.tensor_copy(st[0:1, 400:600], cs[0:1, 0:200])
                nc.vector.tensor_mul(st[0:1, 200:400], st[0:1, 400:600],
                                     st[0:1, 400:600])
                nc.vector.tensor_sub(st[0:1, 200:400], cs[0:1, 200:400],
                                     st[0:1, 200:400])
                nc.scalar.activation(st[0:1, 200:400], st[0:1, 200:400],
                                     AF.Sqrt, bias=_vw1(eps_sb), scale=1.0)
                nc.vector.reciprocal(st[0:1, 200:400], st[0:1, 200:400])
                stb = wk.tile([1, 400], BF16, name=f'stb_{nm}')
                nc.scalar.copy(stb[0:1, 0:200], st[0:1, 400:600])
                nc.scalar.copy(stb[0:1, 200:400], st[0:1, 200:400])
                mb = px.tile([128, CHUNK], F32, name='pxc')[:, 0:200]
                nc.tensor.matmul(mb, ones1r, stb[0:1, 200:400], start=True,
                                 stop=True)
                rbs = wk.tile([128, 200], BF16, name=f'rbs_{nm}')
                nc.scalar.copy(rbs[:, :], mb)
                Pp = pt_pool.tile([128, 200], F32, name='tailps')[0:nout, :]
                nc.tensor.matmul(Pp, Wb, Vb[:, 0:200], start=True, stop=False)
                nc.tensor.matmul(Pp, wsnr, stb[0:1, 0:200], start=False,
                                 stop=True, skip_group_check=True)
                q = wk.tile([128, 200], F32, name=f'q_{nm}')[0:nout, :]
                nc.vector.tensor_mul(q, Pp, rbs[0:nout, :])
                return q

            q1 = ln_lin(pt[:, :], tailb[:, TB_W1B:TB_W1B + 128],
                        tailb[0:1, TB_W1SN:TB_W1SN + 128], 128, 'l1')
            h1p = pt_pool.tile([128, 200], F32, name='tailps')
            nc.scalar.activation(h1p[:, :], q1, AF.Relu, bias=b1p, scale=1.0)
            q2 = ln_lin(h1p[:, :], tailb[:, TB_W2B:TB_W2B + 16],
                        tailb[0:1, TB_W2SN:TB_W2SN + 16], 16, 'l2')
            ob = wk.tile([16, 200], F32, tag='ob')
            nc.scalar.activation(ob[:, :], q2, AF.Identity, bias=b2p, scale=1.0)
            ored = wk.tile([16, 2], F32, tag='ored')
            nc.vector.tensor_reduce(ored[:, :], _vw(ob[:, :], 0, [(100, 2), (1, 100)]),
                                    mybir.AxisListType.X, OP.add)
            for b in range(2):
                nc.sync.dma_start(out=out_d[b, :], in_=ored[:, b:b + 1])
    nc.finalize()
    return nc


_NC_CACHE = {}


def kernel(**inputs):
    inp = {k: np.asarray(v) for k, v in inputs.items()}
    H_ = _host_precompute(inp)
    if 'nc' not in _NC_CACHE:
        _NC_CACHE['nc'] = _build_nc()
    nc = _NC_CACHE['nc']
    in_maps = []
    for c in range(NCORES):
        m = {k: np.ascontiguousarray(H_[k]) for k in H_}
        m.update({k: np.ascontiguousarray(v)
                  for k, v in _host_x(inp, (2 * c, 2 * c + 1)).items()})
        in_maps.append(m)
    res = run_bass_kernel_spmd(nc, in_maps, list(range(NCORES)))
    out = np.zeros((B, N_CLASS), np.float32)
    for c in range(NCORES):
        out[2 * c:2 * c + 2] = res.results[c]['out'][:, :N_CLASS]
    return out

